# revision 65
# baseline (speedup 1.0000x reference)
"""Trainium2 Bass kernel for nn_ConsistencyLoss.

Strategy (pure data-parallel over the agent dim N, 8 cores):
  - Host pads N 20000 -> 20480, shards 2560 agents/core, and builds:
      * tds: per-block [38, 128] f16 stationary tensors holding quantized
        endpoint distances (f16(1024 + 16*dist), exact grid-1 integers)
        plus two ones-rows for the offset-cancel and index-payload terms
      * lp/ln trajectories in a tc-major "gather layout" (bf16): each
        16-partition group owns 320 agents, partitions within a group are
        timestep slots, so the gpsimd indirect-copy gather (whose index
        list is shared across a 16-partition group) can select modes
        per-agent.
  - Match path on device: per-agent scores for all 720 mode permutations
    via two PE matmuls per 128-agent block against a [38, 720] table whose
    rows are -S/16 | +96 | (64+720-p)*2^-17.  Scores are exact f32 on a
    2^-17 grid, so a single max finds the best permutation AND its index
    (packed in the low bits; extracted with f32 offset-rounding).
    Blocks are processed in pairs: 4 matmuls -> one PSUM tile [128,2,1024];
    gpsimd pre-folds 720 -> 360 (TT max) and DVE tensor_reduce finishes,
    splitting the reduction load across both engines.
  - Index -> permutation images via one full-width branchless Lehmer
    decode (tiny f32 ops on DVE), then 4-block gpsimd indirect_copy
    gathers (bitcast to uint32 so the copy moves half the elements).
  - smooth-L1 sums via sum smooth(d) = sum|d| - 0.5*K + 0.5*sum(min(|d|,1)-1)^2:
    DVE TT sub, ACT Abs pass (accum -> sum|d|), DVE 4x-mode min, ACT
    Square(bias=-1) pass (accum).  Zero rows (padding / masked agents)
    contribute exactly 0.
  - The reg loss depends only on pad_loc/pad_loc_target (0.2% of the
    input bytes) and is computed on the host.

Self-contained: hardcodes shapes/sharding; only needs /opt/trn_rl_repo.
"""

import sys
from itertools import permutations

import numpy as np

if "/opt/trn_rl_repo" not in sys.path:
    sys.path.insert(0, "/opt/trn_rl_repo")

NUM_MODES = 6
T = 30
NPERM = 720
N_CORES = 8
PPART = 128
TC = 64  # t*2 coords padded 60 -> 64 (16 slots of 4)

PERMS = np.array(list(permutations(range(NUM_MODES))), dtype=np.int64)  # [720, 6]

EXT_C = 786432.0  # 1.5*2^19: extraction offset (grid 2^-4 over [2^19,2^20))


def _chunks(A):
    """Gather/smooth chunk ranges and per-chunk smooth identity type."""
    if A == 20:
        sch = [(0, 4), (4, 8), (8, 12), (12, 16), (16, 18), (18, 20)]
    else:
        sch = [(c, min(c + 4, A)) for c in range(0, A, 4)]
    ctype = (["A", "A", "A"] + ["D"] * len(sch))[: len(sch)]
    return sch, ctype


def _bf16_pack(x):
    """f32 array -> uint16 bf16 (RNE)."""
    x = np.ascontiguousarray(x, np.float32)
    u = x.view(np.uint32)
    r = ((u >> 16) + ((u >> 15) & 1)).astype(np.uint32)
    return (r & 0xFFFF).astype(np.uint16)


def _host_negs():
    """[38, 720] f16 table: rows 0-35 -S/16, row 36 offset-cancel, row 37 a
    payload encoding (sigma0, sigma1, lehmer d2, d3, d4) of each permutation
    in the low-order score bits."""
    negs = np.zeros((38, NPERM), np.float16)
    for p in range(NPERM):
        for i in range(NUM_MODES):
            negs[i * 6 + PERMS[p, i], p] = np.float16(-1.0 / 16.0)
    negs[36, :] = np.float16(384.0)
    pr = np.arange(NPERM)
    d0 = pr // 120
    r = pr - 120 * d0
    d1 = r // 24
    r = r - 24 * d1
    d2 = r // 6
    r = r - 6 * d2
    d3 = r // 2
    d4 = r - 2 * d3
    k = PERMS[:, 0] * 256 + PERMS[:, 1] * 32 + d2 * 8 + d3 * 2 + d4
    negs[37, :] = ((512.0 + k) * 2.0 ** -16).astype(np.float16)
    return negs


def build_nc(nsh):
    """Per-core Bass program for a shard of `nsh` agents (nsh % 256 == 0)."""
    import concourse.bacc as bacc
    import concourse.mybir as mybir
    import concourse.tile as tile

    f32 = mybir.dt.float32
    f16 = mybir.dt.float16
    bf16 = mybir.dt.bfloat16
    u16 = mybir.dt.uint16
    u32 = mybir.dt.uint32
    i32 = mybir.dt.int32
    Alu = mybir.AluOpType
    Act = mybir.ActivationFunctionType
    AxX = mybir.AxisListType.X

    A = nsh // PPART
    assert A % 4 == 0
    NPAIR = A // 2
    G = 16 * A  # agents per 16-partition group
    UNITS = G * NUM_MODES  # 4-elem units per partition in gather layout
    FREE = UNITS * 4  # bf16 elems per partition

    # chunking of the back half (gather + smooth chunks; smaller at the tail)
    GCH, CTYPE0 = _chunks(A)
    SCH = GCH
    # decode segments (block ranges; must cover gather-chunk boundaries)
    DSEG = [(0, 12), (12, A)] if A == 20 else [(0, A)]
    # lnT/lpT dma chunks
    DCH = [(0, 8), (8, 16), (16, A)] if A == 20 else [(0, A)]

    nc = bacc.Bacc(None, target_bir_lowering=False, debug=False)

    # f16/bf16 payloads are shipped as f32-typed words (bitcast on SBUF side)
    # negs and tds are packed in one dram tensor: [negs | tds] per row
    td_d = nc.declare_dram_parameter("ngtd", [38, (NPERM + A * PPART) // 2], f32, False)
    cc_d = nc.declare_dram_parameter("cconst", [PPART, A], f32, False)
    lnT_d = nc.declare_dram_parameter("lnT", [PPART, FREE // 2], f32, False)
    lpT_d = nc.declare_dram_parameter("lpT", [PPART, FREE // 2], f32, False)
    NACC = 2 * len(SCH)
    out_d = nc.declare_dram_parameter("partials", [PPART, NACC], f32, True)

    with tile.TileContext(nc) as tc:
        with (
            tc.tile_pool(name="cst", bufs=1) as cst,
            tc.tile_pool(name="big", bufs=1) as big,
            tc.tile_pool(name="sml", bufs=1) as sml,
            tc.tile_pool(name="pnm", bufs=2, space="PSUM") as pnm,
        ):
            # ---- small inputs (match-path first: they gate everything);
            # negs + first half of tds land in ONE dma so matmuls start asap ----
            ngtd = cst.tile([38, NPERM + A * PPART], f16)
            negs = ngtd[:, 0:NPERM]
            H = A // 2
            nc.sync.dma_start(
                ngtd[:, 0 : NPERM + H * PPART].bitcast(f32),
                td_d[:, 0 : (NPERM + H * PPART) // 2],
            )
            nc.sync.dma_start(
                ngtd[:, NPERM + H * PPART :].bitcast(f32),
                td_d[:, (NPERM + H * PPART) // 2 :],
            )

            def tds(a):
                return ngtd[:, NPERM + a * PPART : NPERM + (a + 1) * PPART]

            cconst = cst.tile([PPART, A], f32)
            nc.sync.dma_start(cconst[:], cc_d[:])

            # ---- big trajectory tensors (gather layout, bf16), chunked so
            # early gathers / subs can start before the whole load lands ----
            lnT = big.tile([PPART, UNITS, 4], bf16)
            lpT = big.tile([PPART, UNITS, 4], bf16)
            UB = UNITS // A  # units per block (96)
            for lo_, hi_ in DCH:
                nc.sync.dma_start(
                    lnT[:, lo_ * UB : hi_ * UB, :]
                    .rearrange("p a b -> p (a b)")
                    .bitcast(f32),
                    lnT_d[:, lo_ * UB * 2 : hi_ * UB * 2],
                )
            for lo_, hi_ in DCH:
                nc.sync.dma_start(
                    lpT[:, lo_ * UB : hi_ * UB, :]
                    .rearrange("p a b -> p (a b)")
                    .bitcast(f32),
                    lpT_d[:, lo_ * UB * 2 : hi_ * UB * 2],
                )

            partials = sml.tile([PPART, NACC], f32)
            nc.vector.memset(partials[:], 0.0)
            # tiny dummy activation up front so the ACT table set loads
            # during the DMA phase instead of on the smooth critical path
            warm = sml.tile([PPART, 1], bf16)
            nc.vector.memset(warm[:], 0.0)
            nc.scalar.activation(warm[:], warm[:], Act.Abs, bias=0.0)
            nc.scalar.activation(warm[:], warm[:], Act.Square, bias=0.0)
            # PE p-state warmup: junk matmuls keep the PE busy while the tds
            # DMA is in flight so real matmuls run at full clock
            wmm = sml.tile([2, 64], f16)
            nc.vector.memset(wmm[:], 0.0)
            wps = pnm.tile([PPART, 2, 1024], f32, tag="nm")
            for _ in range(6):
                nc.tensor.matmul(wps[0:1, 0, 0:64], wmm[0:1, 0:1], wmm[0:1, :])
            sel = big.tile([PPART, UNITS, 4], bf16)
            dd = big.tile([PPART, FREE], bf16)
            scr = big.tile([PPART, FREE], bf16)
            mseg = sml.tile([PPART, A], f32)
            # one idx tile per decode segment so early gathers only wait on
            # their own segment's writer
            idxt = [
                sml.tile([PPART, hi_ - lo_, 6], u16, name=f"idx{i}")
                for i, (lo_, hi_) in enumerate(DSEG)
            ]

            def idx_ap(a0, a1):
                for (lo_, hi_), t in zip(DSEG, idxt):
                    if lo_ <= a0 and a1 <= hi_:
                        return t[:, a0 - lo_ : a1 - lo_, :]
                raise AssertionError((a0, a1))

            # ---- match: 4 matmuls per block-pair -> PSUM [128, 2, 1024];
            # DVE folds 720->360 out of PSUM (gpsimd cannot read PSUM), the
            # pool folds 360->90 in SBUF, DVE tensor_reduce finishes ----
            def match_pair(j):
                # gpsimd cannot run min/max ops and only one PSUM input is
                # allowed per instruction, so the whole 720-way reduction is
                # a single DVE tensor_reduce straight out of PSUM.
                nm = pnm.tile([PPART, 2, 1024], f32, tag="nm")
                for h in (0, 1):
                    a = 2 * j + h
                    nc.tensor.matmul(nm[:, h, 0:512], tds(a), negs[:, 0:512])
                    nc.tensor.matmul(nm[:, h, 512:NPERM], tds(a), negs[:, 512:NPERM])
                nc.vector.tensor_reduce(
                    mseg[:, 2 * j : 2 * j + 2], nm[:, :, 0:NPERM], AxX, Alu.max
                )

            def decode_stages(lo_, hi_, k, eng=None):
                """Payload extraction + short Lehmer adjust for [lo_, hi_).

                The max value carries (sigma0, sigma1, d2, d3, d4) packed in
                its low bits.  Returns a list of stage thunks so the issue
                order can interleave them with the match stream (DVE runs
                in-order; each stage's ops slot into reduce gaps).
                """
                if eng is None:
                    eng = nc.vector
                L = hi_ - lo_
                vm = mseg[:, lo_:hi_]
                sig = sml.tile([PPART, L, 6], f32, name=f"sig{k}")
                st = {}

                def sslice(i):
                    return sig[:, :, i : i + 1].rearrange("p a x -> p (a x)")

                def tt(op, x, y, nm_, out=None):
                    if out is None:
                        r = sml.tile([PPART, L], f32, name=f"{nm_}_{k}")
                        out = r[:]
                    eng.tensor_tensor(out, x, y, op)
                    return out

                def geadd(v, sv, nm_, out=None):
                    ge = tt(Alu.is_ge, v, sv, nm_ + "g")
                    return tt(Alu.add, v, ge, nm_ + "a", out=out)

                def stage0():
                    c1 = sml.tile([PPART, L], f32, name=f"c1_{k}")
                    nc.vector.tensor_scalar(c1[:], vm, EXT_C, None, Alu.add)
                    negio = sml.tile([PPART, L], f32, name=f"negio{k}")
                    nc.vector.scalar_tensor_tensor(
                        negio[:], c1[:], EXT_C, vm, Alu.subtract, Alu.subtract
                    )
                    nf = sml.tile([PPART, L], i32, name=f"nf{k}")
                    nc.vector.tensor_scalar(
                        nf[:], negio[:], -65536.0, -512.0, Alu.mult, Alu.add
                    )
                    # bit fields: i32-only on DVE (TSP bitVec ops cannot cast)
                    fi = sml.tile([PPART, L, 5], i32, name=f"fi{k}")
                    for j, (shift, mask) in enumerate(
                        ((8, 7), (5, 7), (3, 3), (1, 3), (0, 1))
                    ):
                        nc.vector.tensor_scalar(
                            fi[:, :, j], nf[:], shift, mask,
                            Alu.logical_shift_right, Alu.bitwise_and,
                        )
                    ff = sml.tile([PPART, L, 5], f32, name=f"ff{k}")
                    nc.vector.tensor_copy(ff[:], fi[:])
                    nc.vector.tensor_copy(sig[:, :, 0:2], ff[:, :, 0:2])
                    for n_, i_ in (("s0", 0), ("s1", 1), ("d2", 2), ("d3", 3), ("d4", 4)):
                        st[n_] = ff[:, :, i_ : i_ + 1].rearrange("p a x -> p (a x)")

                def ts(op, x, sc, nm_):
                    r = sml.tile([PPART, L], f32, name=f"{nm_}_{k}")
                    nc.vector.tensor_scalar(r[:], x, sc, None, op)
                    return r[:]

                def rankins(d, ws, nm_, out):
                    # v = d + sum_j [w_j <= d + j - 1]  (w sorted ascending):
                    # shallow parallel form of the branchless rank-insert
                    dps = [d] + [
                        ts(Alu.add, d, float(j), f"{nm_}dp{j}")
                        for j in range(1, len(ws))
                    ]
                    es = [
                        tt(Alu.is_le, w_, dp_, f"{nm_}e{j}")
                        for j, (w_, dp_) in enumerate(zip(ws, dps))
                    ]
                    v = tt(Alu.add, d, es[0], f"{nm_}a0")
                    for j, e_ in enumerate(es[1:], 1):
                        o = out if j == len(es) - 1 else None
                        v = tt(Alu.add, v, e_, f"{nm_}a{j}", out=o)
                    return v

                def stage1():
                    lo01 = tt(Alu.min, st["s0"], st["s1"], "lo")
                    hi01 = tt(Alu.max, st["s0"], st["s1"], "hi")
                    s2 = rankins(st["d2"], [lo01, hi01], "L2", sslice(2))
                    # sorted triple of {s0, s1, s2}
                    t1 = tt(Alu.min, lo01, s2, "t1")
                    t3 = tt(Alu.max, hi01, s2, "t3")
                    m1 = tt(Alu.min, hi01, s2, "m1")
                    tmid = tt(Alu.max, lo01, m1, "tmid")
                    s3 = rankins(st["d3"], [t1, tmid, t3], "L3", sslice(3))
                    st["t1"], st["tmid"], st["t3"], st["s3"] = t1, tmid, t3, s3
                    st["p01"] = tt(Alu.add, st["s0"], st["s1"], "p01")
                    st["p23"] = tt(Alu.add, s2, s3, "p23")

                def stage2():
                    # sorted quad of {t1, tmid, t3, s3}
                    w1 = tt(Alu.min, st["t1"], st["s3"], "w1")
                    q1 = tt(Alu.max, st["t1"], st["s3"], "q1")
                    w2 = tt(Alu.min, st["tmid"], q1, "w2")
                    q2 = tt(Alu.max, st["tmid"], q1, "q2")
                    w3 = tt(Alu.min, st["t3"], q2, "w3")
                    w4 = tt(Alu.max, st["t3"], q2, "w4")
                    s4 = rankins(st["d4"], [w1, w2, w3, w4], "L4", sslice(4))
                    z1 = tt(Alu.add, st["p01"], st["p23"], "z1")
                    z2 = tt(Alu.add, z1, s4, "z2")
                    nc.vector.tensor_scalar(
                        sslice(5), z2, -1.0, 15.0, Alu.mult, Alu.add
                    )
                    # gather index values (uint32 units): 2*sigma + cconst
                    nc.vector.scalar_tensor_tensor(
                        idx_ap(lo_, hi_),
                        sig[:],
                        2.0,
                        cconst[:, lo_:hi_].unsqueeze(2).broadcast_to([PPART, L, 6]),
                        Alu.mult,
                        Alu.add,
                    )

                return [stage0, stage1, stage2]

            def gather(c):
                # 4-block chunks, data/out bitcast to uint32 (half the
                # elements); indices are chunk-local 4B-unit offsets.
                a0, a1 = GCH[c]
                m0, m1 = a0 * UB, a1 * UB
                nc.gpsimd.indirect_copy(
                    sel[:, m0:m1, :]
                    .rearrange("p a b -> p (a b)")
                    .bitcast(u32)
                    .rearrange("p (a b) -> p a b", b=2),
                    lnT[:, m0:m1, :]
                    .rearrange("p a b -> p (a b)")
                    .bitcast(u32)
                    .rearrange("p (a b) -> p a b", b=2),
                    idx_ap(a0, a1).rearrange("p a x -> p (a x)"),
                    True,
                )

            # Per-chunk identity (all chunks): sum smooth(d) = sum|d| - K/2 +
            # 0.5*sum(1-min(|d|,1))^2.  Chunk type picks the engine doing the
            # two nonlinear accum passes: "A" = ACT (Abs / Square), "D" = DVE
            # (stt max(-d,d) / stt square; trn2 DVE has no abs ALU op, but
            # (d mult -1) max d with accum is a legal scalar_tensor_tensor).
            CTYPE = CTYPE0

            def smooth(s):
                a_lo, a_hi = SCH[s]
                e_ = slice(a_lo * UB * 4, a_hi * UB * 4)
                m_ = slice(a_lo * UB, a_hi * UB)
                nc.vector.tensor_sub(
                    dd[:, e_],
                    lpT[:, m_, :].rearrange("p a b -> p (a b)"),
                    sel[:, m_, :].rearrange("p a b -> p (a b)"),
                )
                if CTYPE[s] == "A":
                    nc.scalar.activation(
                        scr[:, e_], dd[:, e_], Act.Abs, bias=0.0,
                        accum_out=partials[:, 2 * s : 2 * s + 1],
                    )
                    # m = min(|d|,1); Square(1-m) back on ACT
                    nc.vector.tensor_scalar(
                        dd[:, e_], scr[:, e_], 1.0, None, Alu.min
                    )
                    nc.scalar.activation(
                        scr[:, e_], dd[:, e_], Act.Square, bias=1.0, scale=-1.0,
                        accum_out=partials[:, 2 * s + 1 : 2 * s + 2],
                    )
                else:
                    nc.vector.scalar_tensor_tensor(
                        scr[:, e_], dd[:, e_], -1.0, dd[:, e_],
                        Alu.mult, Alu.max,
                        accum_out=partials[:, 2 * s : 2 * s + 1],
                    )
                    # m - 1 = min(|d|,1) - 1
                    nc.vector.tensor_scalar(
                        dd[:, e_], scr[:, e_], 1.0, 1.0, Alu.min, Alu.subtract
                    )
                    nc.vector.scalar_tensor_tensor(
                        scr[:, e_], dd[:, e_], 1.0, dd[:, e_],
                        Alu.mult, Alu.mult,
                        accum_out=partials[:, 2 * s + 1 : 2 * s + 2],
                    )

            # ---- issue order.  Cross-engine waits are issue-point coarse
            # (a consumer waits for the producer ENGINE's whole stream up to
            # the consumer's issue position), so: early gathers are issued
            # immediately after decode-segment-0's idx write, late gathers
            # immediately after segment 1, and all smooth work after that.
            # Decode-0 stages interleave with the trailing reduces. ----
            EARLY_G = [
                c for c, (a0, a1) in enumerate(GCH)
                if DSEG[0][0] <= a0 and a1 <= DSEG[0][1]
            ]
            NP1 = DSEG[0][1] // 2  # pairs feeding decode segment 0
            for j in range(NP1):
                match_pair(j)
            d1 = decode_stages(DSEG[0][0], DSEG[0][1], 0)
            d1[0]()
            nj = NP1
            for jr in range(1, len(d1)):
                if nj < NPAIR:
                    match_pair(nj)
                    nj += 1
                d1[jr]()
            for c in EARLY_G:
                gather(c)
            for j in range(nj, NPAIR):
                match_pair(j)
            if len(DSEG) > 1:
                for s_ in decode_stages(DSEG[1][0], DSEG[1][1], 1):
                    s_()
            for c in range(len(GCH)):
                if c not in EARLY_G:
                    gather(c)
            # gate the subs on the last decode's idx so the list scheduler
            # can't starve the decode chain (whose output the last gathers
            # wait on) with ready smooth work
            for c in range(min(3, len(GCH))):
                nc.vector.tensor_scalar(
                    dd[:, GCH[c][0] * UB * 4 : GCH[c][0] * UB * 4 + 1]
                    .bitcast(u16),
                    idxt[-1][:, DSEG[-1][1] - DSEG[-1][0] - 1, 5:6],
                    0,
                    None,
                    Alu.bitwise_and,
                )
            for c in range(len(GCH)):
                smooth(c)

            nc.sync.dma_start(out_d[:], partials[:])

    nc.finalize()
    return nc


def _prep_host(pred_past, pred_now, pad_loc, pad_loc_mask, pad_loc_target, n_pad):
    """Build all per-core host tensors (list of 8 dicts)."""
    n = pred_past.shape[1]
    nsh = n_pad // N_CORES
    A = nsh // PPART

    valid = (~pad_loc_mask).astype(np.float32)

    # full agent-major trajectories, zeroed outside valid agents
    lp = np.zeros((n_pad, NUM_MODES, TC), np.float32)
    ln = np.zeros((n_pad, NUM_MODES, TC), np.float32)
    pp = pred_past[..., :2].transpose(1, 0, 2, 3) + pad_loc.transpose(1, 0, 2)[
        :, :, None, :
    ]
    pn = pred_now[..., :2].transpose(1, 0, 2, 3) + pad_loc_target[:, None, None, :]
    pp *= valid[:, None, None, None]
    pn *= valid[:, None, None, None]
    lp[:n, :, 0:60] = pp.reshape(n, NUM_MODES, 60)
    ln[:n, :, 0:60] = pn.reshape(n, NUM_MODES, 60)

    # quantized endpoint distance matrix -> tds rows (f16, exact ints)
    qd = np.zeros((n_pad, 36), np.float16)
    dx = pp[:, :, None, T - 1, 0] - pn[:, None, :, T - 1, 0]
    dy = pp[:, :, None, T - 1, 1] - pn[:, None, :, T - 1, 1]
    dist = np.minimum(np.sqrt((dx * dx + dy * dy).astype(np.float32)), 8.0)
    qd[:n] = (1024.0 + 16.0 * dist.reshape(n, 36)).astype(np.float16)

    negs = _host_negs()
    # chunk-local uint32-unit gather bases: 2 u32 per (mode, t-slot) unit,
    # 96 units -> 192 u32 per block; base = block offset within its gather
    # chunk (chunks are NOT all 4 blocks: the tail uses 2-block chunks)
    sch, _ = _chunks(A)
    local = np.zeros(A, np.float32)
    for lo, hi in sch:
        local[lo:hi] = np.arange(hi - lo, dtype=np.float32)
    cc = 192.0 * local[None, :] + 12.0 * (
        np.arange(PPART, dtype=np.float32) % 16
    )[:, None]

    in_maps = []
    for c in range(N_CORES):
        s = slice(c * nsh, (c + 1) * nsh)
        lp_c = lp[s]  # [nsh, 6, 64]
        ln_c = ln[s]
        # [a, g, q, i, t, e] -> [g, t, a, q, i, e]
        src_ln = ln_c.reshape(A, 8, 16, NUM_MODES, 16, 4)
        lnT = src_ln.transpose(1, 4, 0, 2, 3, 5).reshape(PPART, -1)
        src_lp = lp_c.reshape(A, 8, 16, NUM_MODES, 16, 4)
        # [g, t, a, i, q, e] so free offset = ((6a+i)*16 + q)*4 + e
        lpT = src_lp.transpose(1, 4, 0, 3, 2, 5).reshape(PPART, -1)

        # ngtd [38, 720 + A*128] f16: negs table, then tds (rows 0-35 =
        # quantized dists transposed, rows 36/37 = 1.0)
        ngtd = np.ones((38, NPERM + A * PPART), np.float16)
        ngtd[:, 0:NPERM] = negs
        ngtd[0:36, NPERM:] = qd[s].reshape(A * PPART, 36).T

        in_maps.append(
            {
                "ngtd": np.ascontiguousarray(ngtd).view(np.uint16).view(np.float32),
                "cconst": cc,
                "lnT": _bf16_pack(lnT).view(np.float32),
                "lpT": _bf16_pack(lpT).view(np.float32),
            }
        )
    return in_maps, float(max(valid.sum(), 1.0)), A


_CACHE = {}
LAST_RESULT = None


def kernel(pred_past, pred_now, pad_loc, pad_loc_mask, pad_loc_target):
    global LAST_RESULT
    from concourse.bass_utils import run_bass_kernel_spmd

    pred_past = np.asarray(pred_past, np.float32)
    pred_now = np.asarray(pred_now, np.float32)
    pad_loc = np.asarray(pad_loc, np.float32)
    pad_loc_mask = np.asarray(pad_loc_mask, bool)
    pad_loc_target = np.asarray(pad_loc_target, np.float32)

    n = pred_past.shape[1]
    step = N_CORES * PPART * 2
    n_pad = ((n + step - 1) // step) * step
    nsh = n_pad // N_CORES

    in_maps, n_valid, A = _prep_host(
        pred_past, pred_now, pad_loc, pad_loc_mask, pad_loc_target, n_pad
    )

    if nsh not in _CACHE:
        _CACHE[nsh] = build_nc(nsh)
    nc = _CACHE[nsh]

    res = run_bass_kernel_spmd(nc, in_maps, list(range(N_CORES)))
    LAST_RESULT = res
    parts = np.stack([r["partials"] for r in res.results])  # [8, 128, ncols]
    sums = parts.sum(axis=(0, 1), dtype=np.float64)

    # all chunks: sum smooth(d) = sum|d| - K/2 + 0.5*sum(1-min(|d|,1))^2
    # (zero rows contribute exactly 0)
    k_cons = N_CORES * PPART * (A * 16 * NUM_MODES * 4)
    cons_sum = sums[0::2].sum() - 0.5 * k_cons + 0.5 * sums[1::2].sum()
    cons_loss = np.float32(cons_sum / (NUM_MODES * T * 2 * n_valid))

    # reg loss is a cheap pure function of two small inputs -> host
    rd = (pad_loc.transpose(1, 0, 2) - pad_loc_target[:, None, :]) * (
        ~pad_loc_mask
    ).astype(np.float32)[:, None, None]
    ra = np.abs(rd)
    rr = np.maximum(1.0 - ra, 0.0)
    reg_sum = (
        ra.sum(dtype=np.float64)
        - 0.5 * rd.size
        + 0.5 * (rr.astype(np.float64) ** 2).sum()
    )
    reg_loss = np.float32(reg_sum / (NUM_MODES * 2 * n_valid))
    return (reg_loss, cons_loss)


# revision 70
# speedup vs baseline: 1.0021x; 1.0021x over previous
"""Trainium2 Bass kernel for nn_ConsistencyLoss.

Strategy (pure data-parallel over the agent dim N, 8 cores):
  - Host pads N 20000 -> 20480, shards 2560 agents/core, and builds:
      * tds: per-block [38, 128] f16 stationary tensors holding quantized
        endpoint distances (f16(1024 + 16*dist), exact grid-1 integers)
        plus two ones-rows for the offset-cancel and index-payload terms
      * lp/ln trajectories in a tc-major "gather layout" (bf16): each
        16-partition group owns 320 agents, partitions within a group are
        timestep slots, so the gpsimd indirect-copy gather (whose index
        list is shared across a 16-partition group) can select modes
        per-agent.
  - Match path on device: per-agent scores for all 720 mode permutations
    via two PE matmuls per 128-agent block against a [38, 720] table whose
    rows are -S/16 | +96 | (64+720-p)*2^-17.  Scores are exact f32 on a
    2^-17 grid, so a single max finds the best permutation AND its index
    (packed in the low bits; extracted with f32 offset-rounding).
    Blocks are processed in pairs: 4 matmuls -> one PSUM tile [128,2,1024];
    gpsimd pre-folds 720 -> 360 (TT max) and DVE tensor_reduce finishes,
    splitting the reduction load across both engines.
  - Index -> permutation images via one full-width branchless Lehmer
    decode (tiny f32 ops on DVE), then 4-block gpsimd indirect_copy
    gathers (bitcast to uint32 so the copy moves half the elements).
  - smooth-L1 sums via sum smooth(d) = sum|d| - 0.5*K + 0.5*sum(min(|d|,1)-1)^2:
    DVE TT sub, ACT Abs pass (accum -> sum|d|), DVE 4x-mode min, ACT
    Square(bias=-1) pass (accum).  Zero rows (padding / masked agents)
    contribute exactly 0.
  - The reg loss depends only on pad_loc/pad_loc_target (0.2% of the
    input bytes) and is computed on the host.

Self-contained: hardcodes shapes/sharding; only needs /opt/trn_rl_repo.
"""

import sys
from itertools import permutations

import numpy as np

if "/opt/trn_rl_repo" not in sys.path:
    sys.path.insert(0, "/opt/trn_rl_repo")

NUM_MODES = 6
T = 30
NPERM = 720
N_CORES = 8
PPART = 128
TC = 64  # t*2 coords padded 60 -> 64 (16 slots of 4)

PERMS = np.array(list(permutations(range(NUM_MODES))), dtype=np.int64)  # [720, 6]

EXT_C = 786432.0  # 1.5*2^19: extraction offset (grid 2^-4 over [2^19,2^20))


def _chunks(A):
    """Gather/smooth chunk ranges and per-chunk smooth identity type."""
    if A == 20:
        sch = [(0, 4), (4, 8), (8, 12), (12, 16), (16, 18), (18, 20)]
    else:
        sch = [(c, min(c + 4, A)) for c in range(0, A, 4)]
    ctype = (["D", "A", "A", "A", "D", "D"] + ["D"] * len(sch))[: len(sch)]
    return sch, ctype


def _bf16_pack(x):
    """f32 array -> uint16 bf16 (RNE)."""
    x = np.ascontiguousarray(x, np.float32)
    u = x.view(np.uint32)
    r = ((u >> 16) + ((u >> 15) & 1)).astype(np.uint32)
    return (r & 0xFFFF).astype(np.uint16)


def _host_negs():
    """[38, 720] f16 table: rows 0-35 -S/16, row 36 offset-cancel, row 37 a
    payload encoding (sigma0, sigma1, lehmer d2, d3, d4) of each permutation
    in the low-order score bits."""
    negs = np.zeros((38, NPERM), np.float16)
    for p in range(NPERM):
        for i in range(NUM_MODES):
            negs[i * 6 + PERMS[p, i], p] = np.float16(-1.0 / 16.0)
    negs[36, :] = np.float16(384.0)
    pr = np.arange(NPERM)
    d0 = pr // 120
    r = pr - 120 * d0
    d1 = r // 24
    r = r - 24 * d1
    d2 = r // 6
    r = r - 6 * d2
    d3 = r // 2
    d4 = r - 2 * d3
    k = PERMS[:, 0] * 256 + PERMS[:, 1] * 32 + d2 * 8 + d3 * 2 + d4
    negs[37, :] = ((512.0 + k) * 2.0 ** -16).astype(np.float16)
    return negs


def build_nc(nsh):
    """Per-core Bass program for a shard of `nsh` agents (nsh % 256 == 0)."""
    import concourse.bacc as bacc
    import concourse.mybir as mybir
    import concourse.tile as tile

    f32 = mybir.dt.float32
    f16 = mybir.dt.float16
    bf16 = mybir.dt.bfloat16
    u16 = mybir.dt.uint16
    u32 = mybir.dt.uint32
    i32 = mybir.dt.int32
    Alu = mybir.AluOpType
    Act = mybir.ActivationFunctionType
    AxX = mybir.AxisListType.X

    A = nsh // PPART
    assert A % 4 == 0
    NPAIR = A // 2
    G = 16 * A  # agents per 16-partition group
    UNITS = G * NUM_MODES  # 4-elem units per partition in gather layout
    FREE = UNITS * 4  # bf16 elems per partition

    # chunking of the back half (gather + smooth chunks; smaller at the tail)
    GCH, CTYPE0 = _chunks(A)
    SCH = GCH
    # decode segments (block ranges; must cover gather-chunk boundaries)
    DSEG = [(0, 12), (12, A)] if A == 20 else [(0, A)]
    # lnT/lpT dma chunks
    DCH = [(0, 8), (8, 16), (16, A)] if A == 20 else [(0, A)]

    nc = bacc.Bacc(None, target_bir_lowering=False, debug=False)

    # f16/bf16 payloads are shipped as f32-typed words (bitcast on SBUF side)
    # negs and tds are packed in one dram tensor: [negs | tds] per row
    td_d = nc.declare_dram_parameter("ngtd", [38, (NPERM + A * PPART) // 2], f32, False)
    cc_d = nc.declare_dram_parameter("cconst", [PPART, A], f32, False)
    lnT_d = nc.declare_dram_parameter("lnT", [PPART, FREE // 2], f32, False)
    lpT_d = nc.declare_dram_parameter("lpT", [PPART, FREE // 2], f32, False)
    NACC = 2 * len(SCH)
    out_d = nc.declare_dram_parameter("partials", [PPART, NACC], f32, True)

    with tile.TileContext(nc) as tc:
        with (
            tc.tile_pool(name="cst", bufs=1) as cst,
            tc.tile_pool(name="big", bufs=1) as big,
            tc.tile_pool(name="sml", bufs=1) as sml,
            tc.tile_pool(name="pnm", bufs=2, space="PSUM") as pnm,
        ):
            # ---- small inputs (match-path first: they gate everything);
            # negs + first half of tds land in ONE dma so matmuls start asap ----
            ngtd = cst.tile([38, NPERM + A * PPART], f16)
            negs = ngtd[:, 0:NPERM]
            H = A // 2
            nc.sync.dma_start(
                ngtd[:, 0 : NPERM + H * PPART].bitcast(f32),
                td_d[:, 0 : (NPERM + H * PPART) // 2],
            )
            nc.sync.dma_start(
                ngtd[:, NPERM + H * PPART :].bitcast(f32),
                td_d[:, (NPERM + H * PPART) // 2 :],
            )

            def tds(a):
                return ngtd[:, NPERM + a * PPART : NPERM + (a + 1) * PPART]

            cconst = cst.tile([PPART, A], f32)
            nc.sync.dma_start(cconst[:], cc_d[:])

            # ---- big trajectory tensors (gather layout, bf16), chunked so
            # early gathers / subs can start before the whole load lands ----
            lnT = big.tile([PPART, UNITS, 4], bf16)
            lpT = big.tile([PPART, UNITS, 4], bf16)
            UB = UNITS // A  # units per block (96)
            for lo_, hi_ in DCH:
                nc.sync.dma_start(
                    lnT[:, lo_ * UB : hi_ * UB, :]
                    .rearrange("p a b -> p (a b)")
                    .bitcast(f32),
                    lnT_d[:, lo_ * UB * 2 : hi_ * UB * 2],
                )
            for lo_, hi_ in DCH:
                nc.sync.dma_start(
                    lpT[:, lo_ * UB : hi_ * UB, :]
                    .rearrange("p a b -> p (a b)")
                    .bitcast(f32),
                    lpT_d[:, lo_ * UB * 2 : hi_ * UB * 2],
                )

            partials = sml.tile([PPART, NACC], f32)
            nc.vector.memset(partials[:], 0.0)
            # tiny dummy activation up front so the ACT table set loads
            # during the DMA phase instead of on the smooth critical path
            warm = sml.tile([PPART, 1], bf16)
            nc.vector.memset(warm[:], 0.0)
            nc.scalar.activation(warm[:], warm[:], Act.Abs, bias=0.0)
            nc.scalar.activation(warm[:], warm[:], Act.Square, bias=0.0)
            # PE p-state warmup: junk matmuls keep the PE busy while the tds
            # DMA is in flight so real matmuls run at full clock
            wmm = sml.tile([2, 64], f16)
            nc.vector.memset(wmm[:], 0.0)
            wps = pnm.tile([PPART, 2, 1024], f32, tag="nm")
            for _ in range(6):
                nc.tensor.matmul(wps[0:1, 0, 0:64], wmm[0:1, 0:1], wmm[0:1, :])
            sel = big.tile([PPART, UNITS, 4], bf16)
            dd = big.tile([PPART, FREE], bf16)
            scr = big.tile([PPART, FREE], bf16)
            mseg = sml.tile([PPART, A], f32)
            # one idx tile per decode segment so early gathers only wait on
            # their own segment's writer
            idxt = [
                sml.tile([PPART, hi_ - lo_, 6], u16, name=f"idx{i}")
                for i, (lo_, hi_) in enumerate(DSEG)
            ]

            def idx_ap(a0, a1):
                for (lo_, hi_), t in zip(DSEG, idxt):
                    if lo_ <= a0 and a1 <= hi_:
                        return t[:, a0 - lo_ : a1 - lo_, :]
                raise AssertionError((a0, a1))

            # ---- match: 4 matmuls per block-pair -> PSUM [128, 2, 1024];
            # DVE folds 720->360 out of PSUM (gpsimd cannot read PSUM), the
            # pool folds 360->90 in SBUF, DVE tensor_reduce finishes ----
            def match_pair(j):
                # gpsimd cannot run min/max ops and only one PSUM input is
                # allowed per instruction, so the whole 720-way reduction is
                # a single DVE tensor_reduce straight out of PSUM.
                nm = pnm.tile([PPART, 2, 1024], f32, tag="nm")
                for h in (0, 1):
                    a = 2 * j + h
                    nc.tensor.matmul(nm[:, h, 0:512], tds(a), negs[:, 0:512])
                    nc.tensor.matmul(nm[:, h, 512:NPERM], tds(a), negs[:, 512:NPERM])
                nc.vector.tensor_reduce(
                    mseg[:, 2 * j : 2 * j + 2], nm[:, :, 0:NPERM], AxX, Alu.max
                )

            def decode_stages(lo_, hi_, k, eng=None):
                """Payload extraction + short Lehmer adjust for [lo_, hi_).

                The max value carries (sigma0, sigma1, d2, d3, d4) packed in
                its low bits.  Returns a list of stage thunks so the issue
                order can interleave them with the match stream (DVE runs
                in-order; each stage's ops slot into reduce gaps).
                """
                if eng is None:
                    eng = nc.vector
                L = hi_ - lo_
                vm = mseg[:, lo_:hi_]
                sig = sml.tile([PPART, L, 6], f32, name=f"sig{k}")
                st = {}

                def sslice(i):
                    return sig[:, :, i : i + 1].rearrange("p a x -> p (a x)")

                def tt(op, x, y, nm_, out=None):
                    if out is None:
                        r = sml.tile([PPART, L], f32, name=f"{nm_}_{k}")
                        out = r[:]
                    eng.tensor_tensor(out, x, y, op)
                    return out

                def geadd(v, sv, nm_, out=None):
                    ge = tt(Alu.is_ge, v, sv, nm_ + "g")
                    return tt(Alu.add, v, ge, nm_ + "a", out=out)

                def stage0():
                    c1 = sml.tile([PPART, L], f32, name=f"c1_{k}")
                    nc.vector.tensor_scalar(c1[:], vm, EXT_C, None, Alu.add)
                    negio = sml.tile([PPART, L], f32, name=f"negio{k}")
                    nc.vector.scalar_tensor_tensor(
                        negio[:], c1[:], EXT_C, vm, Alu.subtract, Alu.subtract
                    )
                    nf = sml.tile([PPART, L], i32, name=f"nf{k}")
                    nc.vector.tensor_scalar(
                        nf[:], negio[:], -65536.0, -512.0, Alu.mult, Alu.add
                    )
                    # bit fields: i32-only on DVE (TSP bitVec ops cannot cast)
                    fi = sml.tile([PPART, L, 5], i32, name=f"fi{k}")
                    for j, (shift, mask) in enumerate(
                        ((8, 7), (5, 7), (3, 3), (1, 3), (0, 1))
                    ):
                        nc.vector.tensor_scalar(
                            fi[:, :, j], nf[:], shift, mask,
                            Alu.logical_shift_right, Alu.bitwise_and,
                        )
                    ff = sml.tile([PPART, L, 5], f32, name=f"ff{k}")
                    nc.vector.tensor_copy(ff[:], fi[:])
                    nc.vector.tensor_copy(sig[:, :, 0:2], ff[:, :, 0:2])
                    for n_, i_ in (("s0", 0), ("s1", 1), ("d2", 2), ("d3", 3), ("d4", 4)):
                        st[n_] = ff[:, :, i_ : i_ + 1].rearrange("p a x -> p (a x)")

                def ts(op, x, sc, nm_):
                    r = sml.tile([PPART, L], f32, name=f"{nm_}_{k}")
                    nc.vector.tensor_scalar(r[:], x, sc, None, op)
                    return r[:]

                def rankins(d, ws, nm_, out):
                    # v = d + sum_j [w_j <= d + j - 1]  (w sorted ascending):
                    # shallow parallel form of the branchless rank-insert
                    dps = [d] + [
                        ts(Alu.add, d, float(j), f"{nm_}dp{j}")
                        for j in range(1, len(ws))
                    ]
                    es = [
                        tt(Alu.is_le, w_, dp_, f"{nm_}e{j}")
                        for j, (w_, dp_) in enumerate(zip(ws, dps))
                    ]
                    v = tt(Alu.add, d, es[0], f"{nm_}a0")
                    for j, e_ in enumerate(es[1:], 1):
                        o = out if j == len(es) - 1 else None
                        v = tt(Alu.add, v, e_, f"{nm_}a{j}", out=o)
                    return v

                def stage1():
                    lo01 = tt(Alu.min, st["s0"], st["s1"], "lo")
                    hi01 = tt(Alu.max, st["s0"], st["s1"], "hi")
                    s2 = rankins(st["d2"], [lo01, hi01], "L2", sslice(2))
                    # sorted triple of {s0, s1, s2}
                    t1 = tt(Alu.min, lo01, s2, "t1")
                    t3 = tt(Alu.max, hi01, s2, "t3")
                    m1 = tt(Alu.min, hi01, s2, "m1")
                    tmid = tt(Alu.max, lo01, m1, "tmid")
                    s3 = rankins(st["d3"], [t1, tmid, t3], "L3", sslice(3))
                    st["t1"], st["tmid"], st["t3"], st["s3"] = t1, tmid, t3, s3
                    st["p01"] = tt(Alu.add, st["s0"], st["s1"], "p01")
                    st["p23"] = tt(Alu.add, s2, s3, "p23")

                def stage2():
                    # sorted quad of {t1, tmid, t3, s3}
                    w1 = tt(Alu.min, st["t1"], st["s3"], "w1")
                    q1 = tt(Alu.max, st["t1"], st["s3"], "q1")
                    w2 = tt(Alu.min, st["tmid"], q1, "w2")
                    q2 = tt(Alu.max, st["tmid"], q1, "q2")
                    w3 = tt(Alu.min, st["t3"], q2, "w3")
                    w4 = tt(Alu.max, st["t3"], q2, "w4")
                    s4 = rankins(st["d4"], [w1, w2, w3, w4], "L4", sslice(4))
                    z1 = tt(Alu.add, st["p01"], st["p23"], "z1")
                    z2 = tt(Alu.add, z1, s4, "z2")
                    nc.vector.tensor_scalar(
                        sslice(5), z2, -1.0, 15.0, Alu.mult, Alu.add
                    )
                    # gather index values (uint32 units): 2*sigma + cconst
                    nc.vector.scalar_tensor_tensor(
                        idx_ap(lo_, hi_),
                        sig[:],
                        2.0,
                        cconst[:, lo_:hi_].unsqueeze(2).broadcast_to([PPART, L, 6]),
                        Alu.mult,
                        Alu.add,
                    )

                return [stage0, stage1, stage2]

            def gather(c):
                # 4-block chunks, data/out bitcast to uint32 (half the
                # elements); indices are chunk-local 4B-unit offsets.
                a0, a1 = GCH[c]
                m0, m1 = a0 * UB, a1 * UB
                nc.gpsimd.indirect_copy(
                    sel[:, m0:m1, :]
                    .rearrange("p a b -> p (a b)")
                    .bitcast(u32)
                    .rearrange("p (a b) -> p a b", b=2),
                    lnT[:, m0:m1, :]
                    .rearrange("p a b -> p (a b)")
                    .bitcast(u32)
                    .rearrange("p (a b) -> p a b", b=2),
                    idx_ap(a0, a1).rearrange("p a x -> p (a x)"),
                    True,
                )

            # Per-chunk identity (all chunks): sum smooth(d) = sum|d| - K/2 +
            # 0.5*sum(1-min(|d|,1))^2.  Chunk type picks the engine doing the
            # two nonlinear accum passes: "A" = ACT (Abs / Square), "D" = DVE
            # (stt max(-d,d) / stt square; trn2 DVE has no abs ALU op, but
            # (d mult -1) max d with accum is a legal scalar_tensor_tensor).
            CTYPE = CTYPE0

            def smooth(s):
                a_lo, a_hi = SCH[s]
                e_ = slice(a_lo * UB * 4, a_hi * UB * 4)
                m_ = slice(a_lo * UB, a_hi * UB)
                nc.vector.tensor_sub(
                    dd[:, e_],
                    lpT[:, m_, :].rearrange("p a b -> p (a b)"),
                    sel[:, m_, :].rearrange("p a b -> p (a b)"),
                )
                if CTYPE[s] == "A":
                    nc.scalar.activation(
                        scr[:, e_], dd[:, e_], Act.Abs, bias=0.0,
                        accum_out=partials[:, 2 * s : 2 * s + 1],
                    )
                    # m = min(|d|,1); Square(1-m) back on ACT
                    nc.vector.tensor_scalar(
                        dd[:, e_], scr[:, e_], 1.0, None, Alu.min
                    )
                    nc.scalar.activation(
                        scr[:, e_], dd[:, e_], Act.Square, bias=1.0, scale=-1.0,
                        accum_out=partials[:, 2 * s + 1 : 2 * s + 2],
                    )
                else:
                    nc.vector.scalar_tensor_tensor(
                        scr[:, e_], dd[:, e_], -1.0, dd[:, e_],
                        Alu.mult, Alu.max,
                        accum_out=partials[:, 2 * s : 2 * s + 1],
                    )
                    # m - 1 = min(|d|,1) - 1
                    nc.vector.tensor_scalar(
                        dd[:, e_], scr[:, e_], 1.0, 1.0, Alu.min, Alu.subtract
                    )
                    nc.vector.scalar_tensor_tensor(
                        scr[:, e_], dd[:, e_], 1.0, dd[:, e_],
                        Alu.mult, Alu.mult,
                        accum_out=partials[:, 2 * s + 1 : 2 * s + 2],
                    )

            # ---- issue order.  Cross-engine waits are issue-point coarse
            # (a consumer waits for the producer ENGINE's whole stream up to
            # the consumer's issue position), so: early gathers are issued
            # immediately after decode-segment-0's idx write, late gathers
            # immediately after segment 1, and all smooth work after that.
            # Decode-0 stages interleave with the trailing reduces. ----
            EARLY_G = [
                c for c, (a0, a1) in enumerate(GCH)
                if DSEG[0][0] <= a0 and a1 <= DSEG[0][1]
            ]
            NP1 = DSEG[0][1] // 2  # pairs feeding decode segment 0
            for j in range(NP1):
                match_pair(j)
            d1 = decode_stages(DSEG[0][0], DSEG[0][1], 0)
            d1[0]()
            nj = NP1
            for jr in range(1, len(d1)):
                if nj < NPAIR:
                    match_pair(nj)
                    nj += 1
                d1[jr]()
            for c in EARLY_G:
                gather(c)
            for j in range(nj, NPAIR):
                match_pair(j)
            if len(DSEG) > 1:
                for s_ in decode_stages(DSEG[1][0], DSEG[1][1], 1):
                    s_()
            for c in range(len(GCH)):
                if c not in EARLY_G:
                    gather(c)
            # gate the subs on the last decode's idx so the list scheduler
            # can't starve the decode chain (whose output the last gathers
            # wait on) with ready smooth work

            for c in range(min(3, len(GCH))):
                nc.vector.tensor_scalar(
                    dd[:, GCH[c][0] * UB * 4 : GCH[c][0] * UB * 4 + 1]
                    .bitcast(u16),
                    idxt[-1][:, DSEG[-1][1] - DSEG[-1][0] - 1, 5:6],
                    0,
                    None,
                    Alu.bitwise_and,
                )
            for c in range(len(GCH)):
                smooth(c)

            nc.sync.dma_start(out_d[:], partials[:])

    nc.finalize()
    return nc


def _prep_host(pred_past, pred_now, pad_loc, pad_loc_mask, pad_loc_target, n_pad):
    """Build all per-core host tensors (list of 8 dicts)."""
    n = pred_past.shape[1]
    nsh = n_pad // N_CORES
    A = nsh // PPART

    valid = (~pad_loc_mask).astype(np.float32)

    # full agent-major trajectories, zeroed outside valid agents
    lp = np.zeros((n_pad, NUM_MODES, TC), np.float32)
    ln = np.zeros((n_pad, NUM_MODES, TC), np.float32)
    pp = pred_past[..., :2].transpose(1, 0, 2, 3) + pad_loc.transpose(1, 0, 2)[
        :, :, None, :
    ]
    pn = pred_now[..., :2].transpose(1, 0, 2, 3) + pad_loc_target[:, None, None, :]
    pp *= valid[:, None, None, None]
    pn *= valid[:, None, None, None]
    lp[:n, :, 0:60] = pp.reshape(n, NUM_MODES, 60)
    ln[:n, :, 0:60] = pn.reshape(n, NUM_MODES, 60)

    # quantized endpoint distance matrix -> tds rows (f16, exact ints)
    qd = np.zeros((n_pad, 36), np.float16)
    dx = pp[:, :, None, T - 1, 0] - pn[:, None, :, T - 1, 0]
    dy = pp[:, :, None, T - 1, 1] - pn[:, None, :, T - 1, 1]
    dist = np.minimum(np.sqrt((dx * dx + dy * dy).astype(np.float32)), 8.0)
    qd[:n] = (1024.0 + 16.0 * dist.reshape(n, 36)).astype(np.float16)

    negs = _host_negs()
    # chunk-local uint32-unit gather bases: 2 u32 per (mode, t-slot) unit,
    # 96 units -> 192 u32 per block; base = block offset within its gather
    # chunk (chunks are NOT all 4 blocks: the tail uses 2-block chunks)
    sch, _ = _chunks(A)
    local = np.zeros(A, np.float32)
    for lo, hi in sch:
        local[lo:hi] = np.arange(hi - lo, dtype=np.float32)
    cc = 192.0 * local[None, :] + 12.0 * (
        np.arange(PPART, dtype=np.float32) % 16
    )[:, None]

    in_maps = []
    for c in range(N_CORES):
        s = slice(c * nsh, (c + 1) * nsh)
        lp_c = lp[s]  # [nsh, 6, 64]
        ln_c = ln[s]
        # [a, g, q, i, t, e] -> [g, t, a, q, i, e]
        src_ln = ln_c.reshape(A, 8, 16, NUM_MODES, 16, 4)
        lnT = src_ln.transpose(1, 4, 0, 2, 3, 5).reshape(PPART, -1)
        src_lp = lp_c.reshape(A, 8, 16, NUM_MODES, 16, 4)
        # [g, t, a, i, q, e] so free offset = ((6a+i)*16 + q)*4 + e
        lpT = src_lp.transpose(1, 4, 0, 3, 2, 5).reshape(PPART, -1)

        # ngtd [38, 720 + A*128] f16: negs table, then tds (rows 0-35 =
        # quantized dists transposed, rows 36/37 = 1.0)
        ngtd = np.ones((38, NPERM + A * PPART), np.float16)
        ngtd[:, 0:NPERM] = negs
        ngtd[0:36, NPERM:] = qd[s].reshape(A * PPART, 36).T

        in_maps.append(
            {
                "ngtd": np.ascontiguousarray(ngtd).view(np.uint16).view(np.float32),
                "cconst": cc,
                "lnT": _bf16_pack(lnT).view(np.float32),
                "lpT": _bf16_pack(lpT).view(np.float32),
            }
        )
    return in_maps, float(max(valid.sum(), 1.0)), A


_CACHE = {}
LAST_RESULT = None


def kernel(pred_past, pred_now, pad_loc, pad_loc_mask, pad_loc_target):
    global LAST_RESULT
    from concourse.bass_utils import run_bass_kernel_spmd

    pred_past = np.asarray(pred_past, np.float32)
    pred_now = np.asarray(pred_now, np.float32)
    pad_loc = np.asarray(pad_loc, np.float32)
    pad_loc_mask = np.asarray(pad_loc_mask, bool)
    pad_loc_target = np.asarray(pad_loc_target, np.float32)

    n = pred_past.shape[1]
    step = N_CORES * PPART * 2
    n_pad = ((n + step - 1) // step) * step
    nsh = n_pad // N_CORES

    in_maps, n_valid, A = _prep_host(
        pred_past, pred_now, pad_loc, pad_loc_mask, pad_loc_target, n_pad
    )

    if nsh not in _CACHE:
        _CACHE[nsh] = build_nc(nsh)
    nc = _CACHE[nsh]

    res = run_bass_kernel_spmd(nc, in_maps, list(range(N_CORES)))
    LAST_RESULT = res
    parts = np.stack([r["partials"] for r in res.results])  # [8, 128, ncols]
    sums = parts.sum(axis=(0, 1), dtype=np.float64)

    # all chunks: sum smooth(d) = sum|d| - K/2 + 0.5*sum(1-min(|d|,1))^2
    # (zero rows contribute exactly 0)
    k_cons = N_CORES * PPART * (A * 16 * NUM_MODES * 4)
    cons_sum = sums[0::2].sum() - 0.5 * k_cons + 0.5 * sums[1::2].sum()
    cons_loss = np.float32(cons_sum / (NUM_MODES * T * 2 * n_valid))

    # reg loss is a cheap pure function of two small inputs -> host
    rd = (pad_loc.transpose(1, 0, 2) - pad_loc_target[:, None, :]) * (
        ~pad_loc_mask
    ).astype(np.float32)[:, None, None]
    ra = np.abs(rd)
    rr = np.maximum(1.0 - ra, 0.0)
    reg_sum = (
        ra.sum(dtype=np.float64)
        - 0.5 * rd.size
        + 0.5 * (rr.astype(np.float64) ** 2).sum()
    )
    reg_loss = np.float32(reg_sum / (NUM_MODES * 2 * n_valid))
    return (reg_loss, cons_loss)


# revision 72
# speedup vs baseline: 1.0297x; 1.0275x over previous
"""Trainium2 Bass kernel for nn_ConsistencyLoss.

Strategy (pure data-parallel over the agent dim N, 8 cores):
  - Host pads N 20000 -> 20480, shards 2560 agents/core, and builds:
      * tds: per-block [38, 128] f16 stationary tensors holding quantized
        endpoint distances (f16(1024 + 16*dist), exact grid-1 integers)
        plus two ones-rows for the offset-cancel and index-payload terms
      * lp/ln trajectories in a tc-major "gather layout" (bf16): each
        16-partition group owns 320 agents, partitions within a group are
        timestep slots, so the gpsimd indirect-copy gather (whose index
        list is shared across a 16-partition group) can select modes
        per-agent.
  - Match path on device: per-agent scores for all 720 mode permutations
    via two PE matmuls per 128-agent block against a [38, 720] table.
    Scores are exact f32 with the permutation's Lehmer code packed in the
    low mantissa bits, so one DVE tensor_reduce over a [128, 2, 720] PSUM
    pair-tile finds the best permutation AND its code (gpsimd cannot read
    PSUM or run min/max, so the reduction is DVE-only).
  - Code -> permutation images via a shallow parallel-rank decode
    (sigma_k = d_k + sum_j [w_j <= d_k + j - 1]) in two segments that
    interleave with the trailing reduces; then gpsimd indirect_copy
    gathers (data bitcast to uint32 so the copy moves half the elements),
    issued immediately after their segment's idx write.
  - smooth-L1 sums via sum smooth(d) = sum|d| - K/2 + 0.5*sum(1-min(|d|,1))^2
    per chunk: DVE TT sub, then either ACT Abs/Square accum passes
    ("A" chunks) or DVE stt max(-d,d)/square accum passes ("D" chunks;
    trn2 DVE has no abs ALU op, but (d mult -1) max d is legal stt).
    The A/D mix balances the DVE and ACT tails.  Zero rows (padding /
    masked agents) contribute exactly 0.
  - The reg loss depends only on pad_loc/pad_loc_target (0.2% of the
    input bytes) and is computed on the host.

Self-contained: hardcodes shapes/sharding; only needs /opt/trn_rl_repo.
"""

import sys
from itertools import permutations

import numpy as np

if "/opt/trn_rl_repo" not in sys.path:
    sys.path.insert(0, "/opt/trn_rl_repo")

NUM_MODES = 6
T = 30
NPERM = 720
N_CORES = 8
PPART = 128
TC = 64  # t*2 coords padded 60 -> 64 (16 slots of 4)

PERMS = np.array(list(permutations(range(NUM_MODES))), dtype=np.int64)  # [720, 6]

EXT_C = 786432.0  # 1.5*2^19: extraction offset (grid 2^-4 over [2^19,2^20))


def _chunks(A):
    """Gather/smooth chunk ranges and per-chunk smooth identity type."""
    if A == 20:
        sch = [(0, 4), (4, 8), (8, 12), (12, 16), (16, 18), (18, 20)]
    else:
        sch = [(c, min(c + 4, A)) for c in range(0, A, 4)]
    ctype = (["D", "A", "A", "A", "D", "D"] + ["D"] * len(sch))[: len(sch)]
    return sch, ctype


def _bf16_pack(x):
    """f32 array -> uint16 bf16 (RNE)."""
    x = np.ascontiguousarray(x, np.float32)
    u = x.view(np.uint32)
    r = ((u >> 16) + ((u >> 15) & 1)).astype(np.uint32)
    return (r & 0xFFFF).astype(np.uint16)


def _host_negs():
    """[38, 720] f16 table: rows 0-35 -S/16, row 36 offset-cancel, row 37 a
    payload encoding (sigma0, sigma1, lehmer d2, d3, d4) of each permutation
    in the low-order score bits."""
    negs = np.zeros((38, NPERM), np.float16)
    for p in range(NPERM):
        for i in range(NUM_MODES):
            negs[i * 6 + PERMS[p, i], p] = np.float16(-1.0 / 16.0)
    negs[36, :] = np.float16(384.0)
    pr = np.arange(NPERM)
    d0 = pr // 120
    r = pr - 120 * d0
    d1 = r // 24
    r = r - 24 * d1
    d2 = r // 6
    r = r - 6 * d2
    d3 = r // 2
    d4 = r - 2 * d3
    k = PERMS[:, 0] * 256 + PERMS[:, 1] * 32 + d2 * 8 + d3 * 2 + d4
    negs[37, :] = ((512.0 + k) * 2.0 ** -16).astype(np.float16)
    return negs


def build_nc(nsh):
    """Per-core Bass program for a shard of `nsh` agents (nsh % 256 == 0)."""
    import concourse.bacc as bacc
    import concourse.mybir as mybir
    import concourse.tile as tile

    f32 = mybir.dt.float32
    f16 = mybir.dt.float16
    bf16 = mybir.dt.bfloat16
    u16 = mybir.dt.uint16
    u32 = mybir.dt.uint32
    i32 = mybir.dt.int32
    Alu = mybir.AluOpType
    Act = mybir.ActivationFunctionType
    AxX = mybir.AxisListType.X

    A = nsh // PPART
    assert A % 4 == 0
    NPAIR = A // 2
    G = 16 * A  # agents per 16-partition group
    UNITS = G * NUM_MODES  # 4-elem units per partition in gather layout
    FREE = UNITS * 4  # bf16 elems per partition

    # chunking of the back half (gather + smooth chunks; smaller at the tail)
    GCH, CTYPE0 = _chunks(A)
    SCH = GCH
    # decode segments (block ranges; must cover gather-chunk boundaries)
    DSEG = [(0, A)]
    # lnT/lpT dma chunks
    DCH = [(0, 8), (8, 16), (16, A)] if A == 20 else [(0, A)]

    nc = bacc.Bacc(None, target_bir_lowering=False, debug=False)

    # f16/bf16 payloads are shipped as f32-typed words (bitcast on SBUF side)
    # negs and tds are packed in one dram tensor: [negs | tds] per row
    td_d = nc.declare_dram_parameter("ngtd", [38, (NPERM + A * PPART) // 2], f32, False)
    cc_d = nc.declare_dram_parameter("cconst", [PPART, A], f32, False)
    lnT_d = nc.declare_dram_parameter("lnT", [PPART, FREE // 2], f32, False)
    lpT_d = nc.declare_dram_parameter("lpT", [PPART, FREE // 2], f32, False)
    NACC = 2 * len(SCH)
    out_d = nc.declare_dram_parameter("partials", [PPART, NACC], f32, True)

    with tile.TileContext(nc) as tc:
        with (
            tc.tile_pool(name="cst", bufs=1) as cst,
            tc.tile_pool(name="big", bufs=1) as big,
            tc.tile_pool(name="sml", bufs=1) as sml,
            tc.tile_pool(name="pnm", bufs=2, space="PSUM") as pnm,
        ):
            # ---- small inputs (match-path first: they gate everything);
            # negs + first half of tds land in ONE dma so matmuls start asap ----
            ngtd = cst.tile([38, NPERM + A * PPART], f16)
            negs = ngtd[:, 0:NPERM]
            H = A // 2
            nc.sync.dma_start(
                ngtd[:, 0 : NPERM + H * PPART].bitcast(f32),
                td_d[:, 0 : (NPERM + H * PPART) // 2],
            )
            nc.sync.dma_start(
                ngtd[:, NPERM + H * PPART :].bitcast(f32),
                td_d[:, (NPERM + H * PPART) // 2 :],
            )

            def tds(a):
                return ngtd[:, NPERM + a * PPART : NPERM + (a + 1) * PPART]

            cconst = cst.tile([PPART, A], f32)
            nc.sync.dma_start(cconst[:], cc_d[:])

            # ---- big trajectory tensors (gather layout, bf16), chunked so
            # early gathers / subs can start before the whole load lands ----
            lnT = big.tile([PPART, UNITS, 4], bf16)
            lpT = big.tile([PPART, UNITS, 4], bf16)
            UB = UNITS // A  # units per block (96)
            for lo_, hi_ in DCH:
                nc.sync.dma_start(
                    lnT[:, lo_ * UB : hi_ * UB, :]
                    .rearrange("p a b -> p (a b)")
                    .bitcast(f32),
                    lnT_d[:, lo_ * UB * 2 : hi_ * UB * 2],
                )
            for lo_, hi_ in DCH:
                nc.sync.dma_start(
                    lpT[:, lo_ * UB : hi_ * UB, :]
                    .rearrange("p a b -> p (a b)")
                    .bitcast(f32),
                    lpT_d[:, lo_ * UB * 2 : hi_ * UB * 2],
                )

            partials = sml.tile([PPART, NACC], f32)
            nc.vector.memset(partials[:], 0.0)
            # tiny dummy activation up front so the ACT table set loads
            # during the DMA phase instead of on the smooth critical path
            warm = sml.tile([PPART, 1], bf16)
            nc.vector.memset(warm[:], 0.0)
            nc.scalar.activation(warm[:], warm[:], Act.Abs, bias=0.0)
            nc.scalar.activation(warm[:], warm[:], Act.Square, bias=0.0)
            # PE p-state warmup: junk matmuls keep the PE busy while the tds
            # DMA is in flight so real matmuls run at full clock
            wmm = sml.tile([2, 64], f16)
            nc.vector.memset(wmm[:], 0.0)
            wps = pnm.tile([PPART, 2, 1024], f32, tag="nm")
            for _ in range(6):
                nc.tensor.matmul(wps[0:1, 0, 0:64], wmm[0:1, 0:1], wmm[0:1, :])
            sel = big.tile([PPART, UNITS, 4], bf16)
            dd = big.tile([PPART, FREE], bf16)
            scr = big.tile([PPART, FREE], bf16)
            mseg = sml.tile([PPART, A], f32)
            # one idx tile per decode segment so early gathers only wait on
            # their own segment's writer
            idxt = [
                sml.tile([PPART, hi_ - lo_, 6], u16, name=f"idx{i}")
                for i, (lo_, hi_) in enumerate(DSEG)
            ]

            def idx_ap(a0, a1):
                for (lo_, hi_), t in zip(DSEG, idxt):
                    if lo_ <= a0 and a1 <= hi_:
                        return t[:, a0 - lo_ : a1 - lo_, :]
                raise AssertionError((a0, a1))

            # ---- match: 4 matmuls per block-pair -> PSUM [128, 2, 1024];
            # DVE folds 720->360 out of PSUM (gpsimd cannot read PSUM), the
            # pool folds 360->90 in SBUF, DVE tensor_reduce finishes ----
            def match_pair(j):
                # gpsimd cannot run min/max ops and only one PSUM input is
                # allowed per instruction, so the whole 720-way reduction is
                # a single DVE tensor_reduce straight out of PSUM.
                nm = pnm.tile([PPART, 2, 1024], f32, tag="nm")
                for h in (0, 1):
                    a = 2 * j + h
                    nc.tensor.matmul(nm[:, h, 0:512], tds(a), negs[:, 0:512])
                    nc.tensor.matmul(nm[:, h, 512:NPERM], tds(a), negs[:, 512:NPERM])
                nc.vector.tensor_reduce(
                    mseg[:, 2 * j : 2 * j + 2], nm[:, :, 0:NPERM], AxX, Alu.max
                )

            def decode_stages(lo_, hi_, k, eng=None):
                """Payload extraction + short Lehmer adjust for [lo_, hi_).

                The max value carries (sigma0, sigma1, d2, d3, d4) packed in
                its low bits.  Returns a list of stage thunks so the issue
                order can interleave them with the match stream (DVE runs
                in-order; each stage's ops slot into reduce gaps).
                """
                if eng is None:
                    eng = nc.vector
                L = hi_ - lo_
                vm = mseg[:, lo_:hi_]
                sig = sml.tile([PPART, L, 6], f32, name=f"sig{k}")
                st = {}

                def sslice(i):
                    return sig[:, :, i : i + 1].rearrange("p a x -> p (a x)")

                def tt(op, x, y, nm_, out=None):
                    if out is None:
                        r = sml.tile([PPART, L], f32, name=f"{nm_}_{k}")
                        out = r[:]
                    eng.tensor_tensor(out, x, y, op)
                    return out

                def geadd(v, sv, nm_, out=None):
                    ge = tt(Alu.is_ge, v, sv, nm_ + "g")
                    return tt(Alu.add, v, ge, nm_ + "a", out=out)

                def stage0():
                    c1 = sml.tile([PPART, L], f32, name=f"c1_{k}")
                    nc.vector.tensor_scalar(c1[:], vm, EXT_C, None, Alu.add)
                    negio = sml.tile([PPART, L], f32, name=f"negio{k}")
                    nc.vector.scalar_tensor_tensor(
                        negio[:], c1[:], EXT_C, vm, Alu.subtract, Alu.subtract
                    )
                    nf = sml.tile([PPART, L], i32, name=f"nf{k}")
                    nc.vector.tensor_scalar(
                        nf[:], negio[:], -65536.0, -512.0, Alu.mult, Alu.add
                    )
                    # bit fields: i32-only on DVE (TSP bitVec ops cannot cast)
                    fi = sml.tile([PPART, L, 5], i32, name=f"fi{k}")
                    for j, (shift, mask) in enumerate(
                        ((8, 7), (5, 7), (3, 3), (1, 3), (0, 1))
                    ):
                        nc.vector.tensor_scalar(
                            fi[:, :, j], nf[:], shift, mask,
                            Alu.logical_shift_right, Alu.bitwise_and,
                        )
                    ff = sml.tile([PPART, L, 5], f32, name=f"ff{k}")
                    nc.vector.tensor_copy(ff[:], fi[:])
                    nc.vector.tensor_copy(sig[:, :, 0:2], ff[:, :, 0:2])
                    for n_, i_ in (("s0", 0), ("s1", 1), ("d2", 2), ("d3", 3), ("d4", 4)):
                        st[n_] = ff[:, :, i_ : i_ + 1].rearrange("p a x -> p (a x)")

                def ts(op, x, sc, nm_):
                    r = sml.tile([PPART, L], f32, name=f"{nm_}_{k}")
                    nc.vector.tensor_scalar(r[:], x, sc, None, op)
                    return r[:]

                def rankins(d, ws, nm_, out):
                    # v = d + sum_j [w_j <= d + j - 1]  (w sorted ascending):
                    # shallow parallel form of the branchless rank-insert
                    dps = [d] + [
                        ts(Alu.add, d, float(j), f"{nm_}dp{j}")
                        for j in range(1, len(ws))
                    ]
                    es = [
                        tt(Alu.is_le, w_, dp_, f"{nm_}e{j}")
                        for j, (w_, dp_) in enumerate(zip(ws, dps))
                    ]
                    v = tt(Alu.add, d, es[0], f"{nm_}a0")
                    for j, e_ in enumerate(es[1:], 1):
                        o = out if j == len(es) - 1 else None
                        v = tt(Alu.add, v, e_, f"{nm_}a{j}", out=o)
                    return v

                def stage1():
                    lo01 = tt(Alu.min, st["s0"], st["s1"], "lo")
                    hi01 = tt(Alu.max, st["s0"], st["s1"], "hi")
                    s2 = rankins(st["d2"], [lo01, hi01], "L2", sslice(2))
                    # sorted triple of {s0, s1, s2}
                    t1 = tt(Alu.min, lo01, s2, "t1")
                    t3 = tt(Alu.max, hi01, s2, "t3")
                    m1 = tt(Alu.min, hi01, s2, "m1")
                    tmid = tt(Alu.max, lo01, m1, "tmid")
                    s3 = rankins(st["d3"], [t1, tmid, t3], "L3", sslice(3))
                    st["t1"], st["tmid"], st["t3"], st["s3"] = t1, tmid, t3, s3
                    st["p01"] = tt(Alu.add, st["s0"], st["s1"], "p01")
                    st["p23"] = tt(Alu.add, s2, s3, "p23")

                def stage2():
                    # sorted quad of {t1, tmid, t3, s3}
                    w1 = tt(Alu.min, st["t1"], st["s3"], "w1")
                    q1 = tt(Alu.max, st["t1"], st["s3"], "q1")
                    w2 = tt(Alu.min, st["tmid"], q1, "w2")
                    q2 = tt(Alu.max, st["tmid"], q1, "q2")
                    w3 = tt(Alu.min, st["t3"], q2, "w3")
                    w4 = tt(Alu.max, st["t3"], q2, "w4")
                    s4 = rankins(st["d4"], [w1, w2, w3, w4], "L4", sslice(4))
                    z1 = tt(Alu.add, st["p01"], st["p23"], "z1")
                    z2 = tt(Alu.add, z1, s4, "z2")
                    nc.vector.tensor_scalar(
                        sslice(5), z2, -1.0, 15.0, Alu.mult, Alu.add
                    )
                    # gather index values (uint32 units): 2*sigma + cconst
                    nc.vector.scalar_tensor_tensor(
                        idx_ap(lo_, hi_),
                        sig[:],
                        2.0,
                        cconst[:, lo_:hi_].unsqueeze(2).broadcast_to([PPART, L, 6]),
                        Alu.mult,
                        Alu.add,
                    )

                return [stage0, stage1, stage2]

            def gather(c):
                # 4-block chunks, data/out bitcast to uint32 (half the
                # elements); indices are chunk-local 4B-unit offsets.
                a0, a1 = GCH[c]
                m0, m1 = a0 * UB, a1 * UB
                nc.gpsimd.indirect_copy(
                    sel[:, m0:m1, :]
                    .rearrange("p a b -> p (a b)")
                    .bitcast(u32)
                    .rearrange("p (a b) -> p a b", b=2),
                    lnT[:, m0:m1, :]
                    .rearrange("p a b -> p (a b)")
                    .bitcast(u32)
                    .rearrange("p (a b) -> p a b", b=2),
                    idx_ap(a0, a1).rearrange("p a x -> p (a x)"),
                    True,
                )

            # Per-chunk identity (all chunks): sum smooth(d) = sum|d| - K/2 +
            # 0.5*sum(1-min(|d|,1))^2.  Chunk type picks the engine doing the
            # two nonlinear accum passes: "A" = ACT (Abs / Square), "D" = DVE
            # (stt max(-d,d) / stt square; trn2 DVE has no abs ALU op, but
            # (d mult -1) max d with accum is a legal scalar_tensor_tensor).
            CTYPE = CTYPE0

            def smooth(s):
                a_lo, a_hi = SCH[s]
                e_ = slice(a_lo * UB * 4, a_hi * UB * 4)
                m_ = slice(a_lo * UB, a_hi * UB)
                nc.vector.tensor_sub(
                    dd[:, e_],
                    lpT[:, m_, :].rearrange("p a b -> p (a b)"),
                    sel[:, m_, :].rearrange("p a b -> p (a b)"),
                )
                if CTYPE[s] == "A":
                    nc.scalar.activation(
                        scr[:, e_], dd[:, e_], Act.Abs, bias=0.0,
                        accum_out=partials[:, 2 * s : 2 * s + 1],
                    )
                    # m = min(|d|,1); Square(1-m) back on ACT
                    nc.vector.tensor_scalar(
                        dd[:, e_], scr[:, e_], 1.0, None, Alu.min
                    )
                    nc.scalar.activation(
                        scr[:, e_], dd[:, e_], Act.Square, bias=1.0, scale=-1.0,
                        accum_out=partials[:, 2 * s + 1 : 2 * s + 2],
                    )
                else:
                    nc.vector.scalar_tensor_tensor(
                        scr[:, e_], dd[:, e_], -1.0, dd[:, e_],
                        Alu.mult, Alu.max,
                        accum_out=partials[:, 2 * s : 2 * s + 1],
                    )
                    # m - 1 = min(|d|,1) - 1
                    nc.vector.tensor_scalar(
                        dd[:, e_], scr[:, e_], 1.0, 1.0, Alu.min, Alu.subtract
                    )
                    nc.vector.scalar_tensor_tensor(
                        scr[:, e_], dd[:, e_], 1.0, dd[:, e_],
                        Alu.mult, Alu.mult,
                        accum_out=partials[:, 2 * s + 1 : 2 * s + 2],
                    )

            # ---- issue order.  Cross-engine waits are issue-point coarse
            # (a consumer waits for the producer ENGINE's whole stream up to
            # the consumer's issue position), so: early gathers are issued
            # immediately after decode-segment-0's idx write, late gathers
            # immediately after segment 1, and all smooth work after that.
            # Decode-0 stages interleave with the trailing reduces. ----
            EARLY_G = [
                c for c, (a0, a1) in enumerate(GCH)
                if DSEG[0][0] <= a0 and a1 <= DSEG[0][1]
            ]
            NP1 = DSEG[0][1] // 2  # pairs feeding decode segment 0
            for j in range(NP1):
                match_pair(j)
            d1 = decode_stages(DSEG[0][0], DSEG[0][1], 0)
            d1[0]()
            nj = NP1
            for jr in range(1, len(d1)):
                if nj < NPAIR:
                    match_pair(nj)
                    nj += 1
                d1[jr]()
            for c in EARLY_G:
                gather(c)
            for j in range(nj, NPAIR):
                match_pair(j)
            if len(DSEG) > 1:
                for s_ in decode_stages(DSEG[1][0], DSEG[1][1], 1):
                    s_()
            for c in range(len(GCH)):
                if c not in EARLY_G:
                    gather(c)
            # gate the subs on the last decode's idx so the list scheduler
            # can't starve the decode chain (whose output the last gathers
            # wait on) with ready smooth work

            for c in range(min(3, len(GCH))):
                nc.vector.tensor_scalar(
                    dd[:, GCH[c][0] * UB * 4 : GCH[c][0] * UB * 4 + 1]
                    .bitcast(u16),
                    idxt[-1][:, DSEG[-1][1] - DSEG[-1][0] - 1, 5:6],
                    0,
                    None,
                    Alu.bitwise_and,
                )
            for c in range(len(GCH)):
                smooth(c)

            nc.sync.dma_start(out_d[:], partials[:])

    nc.finalize()
    return nc


def _prep_host(pred_past, pred_now, pad_loc, pad_loc_mask, pad_loc_target, n_pad):
    """Build all per-core host tensors (list of 8 dicts)."""
    n = pred_past.shape[1]
    nsh = n_pad // N_CORES
    A = nsh // PPART

    valid = (~pad_loc_mask).astype(np.float32)

    # full agent-major trajectories, zeroed outside valid agents
    lp = np.zeros((n_pad, NUM_MODES, TC), np.float32)
    ln = np.zeros((n_pad, NUM_MODES, TC), np.float32)
    pp = pred_past[..., :2].transpose(1, 0, 2, 3) + pad_loc.transpose(1, 0, 2)[
        :, :, None, :
    ]
    pn = pred_now[..., :2].transpose(1, 0, 2, 3) + pad_loc_target[:, None, None, :]
    pp *= valid[:, None, None, None]
    pn *= valid[:, None, None, None]
    lp[:n, :, 0:60] = pp.reshape(n, NUM_MODES, 60)
    ln[:n, :, 0:60] = pn.reshape(n, NUM_MODES, 60)

    # quantized endpoint distance matrix -> tds rows (f16, exact ints)
    qd = np.zeros((n_pad, 36), np.float16)
    dx = pp[:, :, None, T - 1, 0] - pn[:, None, :, T - 1, 0]
    dy = pp[:, :, None, T - 1, 1] - pn[:, None, :, T - 1, 1]
    dist = np.minimum(np.sqrt((dx * dx + dy * dy).astype(np.float32)), 8.0)
    qd[:n] = (1024.0 + 16.0 * dist.reshape(n, 36)).astype(np.float16)

    negs = _host_negs()
    # chunk-local uint32-unit gather bases: 2 u32 per (mode, t-slot) unit,
    # 96 units -> 192 u32 per block; base = block offset within its gather
    # chunk (chunks are NOT all 4 blocks: the tail uses 2-block chunks)
    sch, _ = _chunks(A)
    local = np.zeros(A, np.float32)
    for lo, hi in sch:
        local[lo:hi] = np.arange(hi - lo, dtype=np.float32)
    cc = 192.0 * local[None, :] + 12.0 * (
        np.arange(PPART, dtype=np.float32) % 16
    )[:, None]

    in_maps = []
    for c in range(N_CORES):
        s = slice(c * nsh, (c + 1) * nsh)
        lp_c = lp[s]  # [nsh, 6, 64]
        ln_c = ln[s]
        # [a, g, q, i, t, e] -> [g, t, a, q, i, e]
        src_ln = ln_c.reshape(A, 8, 16, NUM_MODES, 16, 4)
        lnT = src_ln.transpose(1, 4, 0, 2, 3, 5).reshape(PPART, -1)
        src_lp = lp_c.reshape(A, 8, 16, NUM_MODES, 16, 4)
        # [g, t, a, i, q, e] so free offset = ((6a+i)*16 + q)*4 + e
        lpT = src_lp.transpose(1, 4, 0, 3, 2, 5).reshape(PPART, -1)

        # ngtd [38, 720 + A*128] f16: negs table, then tds (rows 0-35 =
        # quantized dists transposed, rows 36/37 = 1.0)
        ngtd = np.ones((38, NPERM + A * PPART), np.float16)
        ngtd[:, 0:NPERM] = negs
        ngtd[0:36, NPERM:] = qd[s].reshape(A * PPART, 36).T

        in_maps.append(
            {
                "ngtd": np.ascontiguousarray(ngtd).view(np.uint16).view(np.float32),
                "cconst": cc,
                "lnT": _bf16_pack(lnT).view(np.float32),
                "lpT": _bf16_pack(lpT).view(np.float32),
            }
        )
    return in_maps, float(max(valid.sum(), 1.0)), A


_CACHE = {}
LAST_RESULT = None


def kernel(pred_past, pred_now, pad_loc, pad_loc_mask, pad_loc_target):
    global LAST_RESULT
    from concourse.bass_utils import run_bass_kernel_spmd

    pred_past = np.asarray(pred_past, np.float32)
    pred_now = np.asarray(pred_now, np.float32)
    pad_loc = np.asarray(pad_loc, np.float32)
    pad_loc_mask = np.asarray(pad_loc_mask, bool)
    pad_loc_target = np.asarray(pad_loc_target, np.float32)

    n = pred_past.shape[1]
    step = N_CORES * PPART * 2
    n_pad = ((n + step - 1) // step) * step
    nsh = n_pad // N_CORES

    in_maps, n_valid, A = _prep_host(
        pred_past, pred_now, pad_loc, pad_loc_mask, pad_loc_target, n_pad
    )

    if nsh not in _CACHE:
        _CACHE[nsh] = build_nc(nsh)
    nc = _CACHE[nsh]

    res = run_bass_kernel_spmd(nc, in_maps, list(range(N_CORES)))
    LAST_RESULT = res
    parts = np.stack([r["partials"] for r in res.results])  # [8, 128, ncols]
    sums = parts.sum(axis=(0, 1), dtype=np.float64)

    # all chunks: sum smooth(d) = sum|d| - K/2 + 0.5*sum(1-min(|d|,1))^2
    # (zero rows contribute exactly 0)
    k_cons = N_CORES * PPART * (A * 16 * NUM_MODES * 4)
    cons_sum = sums[0::2].sum() - 0.5 * k_cons + 0.5 * sums[1::2].sum()
    cons_loss = np.float32(cons_sum / (NUM_MODES * T * 2 * n_valid))

    # reg loss is a cheap pure function of two small inputs -> host
    rd = (pad_loc.transpose(1, 0, 2) - pad_loc_target[:, None, :]) * (
        ~pad_loc_mask
    ).astype(np.float32)[:, None, None]
    ra = np.abs(rd)
    rr = np.maximum(1.0 - ra, 0.0)
    reg_sum = (
        ra.sum(dtype=np.float64)
        - 0.5 * rd.size
        + 0.5 * (rr.astype(np.float64) ** 2).sum()
    )
    reg_loss = np.float32(reg_sum / (NUM_MODES * 2 * n_valid))
    return (reg_loss, cons_loss)


# revision 74
# speedup vs baseline: 1.0334x; 1.0037x over previous
"""Trainium2 Bass kernel for nn_ConsistencyLoss.

Strategy (pure data-parallel over the agent dim N, 8 cores):
  - Host pads N 20000 -> 20480, shards 2560 agents/core, and builds:
      * tds: per-block [38, 128] f16 stationary tensors holding quantized
        endpoint distances (f16(1024 + 16*dist), exact grid-1 integers)
        plus two ones-rows for the offset-cancel and index-payload terms
      * lp/ln trajectories in a tc-major "gather layout" (bf16): each
        16-partition group owns 320 agents, partitions within a group are
        timestep slots, so the gpsimd indirect-copy gather (whose index
        list is shared across a 16-partition group) can select modes
        per-agent.
  - Match path on device: per-agent scores for all 720 mode permutations
    via two PE matmuls per 128-agent block against a [38, 720] table.
    Scores are exact f32 with the permutation's Lehmer code packed in the
    low mantissa bits, so one DVE tensor_reduce over a [128, 2, 720] PSUM
    pair-tile finds the best permutation AND its code (gpsimd cannot read
    PSUM or run min/max, so the reduction is DVE-only).
  - Code -> permutation images via a shallow parallel-rank decode
    (sigma_k = d_k + sum_j [w_j <= d_k + j - 1]) in one full-width batch
    after the last reduce; then gpsimd indirect_copy gathers (data bitcast
    to uint32 so the copy moves half the elements).
  - smooth-L1 sums via sum smooth(d) = sum|d| - K/2 + 0.5*sum(1-min(|d|,1))^2
    per chunk: DVE TT sub, then either ACT Abs/Square accum passes
    ("A" chunks) or DVE stt max(-d,d)/square accum passes ("D" chunks;
    trn2 DVE has no abs ALU op, but (d mult -1) max d is legal stt).
    The A/D mix balances the DVE and ACT tails.  Zero rows (padding /
    masked agents) contribute exactly 0.
  - The reg loss depends only on pad_loc/pad_loc_target (0.2% of the
    input bytes) and is computed on the host.

Self-contained: hardcodes shapes/sharding; only needs /opt/trn_rl_repo.
"""

import sys
from itertools import permutations

import numpy as np

if "/opt/trn_rl_repo" not in sys.path:
    sys.path.insert(0, "/opt/trn_rl_repo")

NUM_MODES = 6
T = 30
NPERM = 720
N_CORES = 8
PPART = 128
TC = 64  # t*2 coords padded 60 -> 64 (16 slots of 4)

PERMS = np.array(list(permutations(range(NUM_MODES))), dtype=np.int64)  # [720, 6]

EXT_C = 786432.0  # 1.5*2^19: extraction offset (grid 2^-4 over [2^19,2^20))


def _chunks(A):
    """Gather/smooth chunk ranges and per-chunk smooth identity type."""
    if A == 20:
        sch = [(0, 4), (4, 8), (8, 12), (12, 16), (16, 18), (18, 20)]
    else:
        sch = [(c, min(c + 4, A)) for c in range(0, A, 4)]
    ctype = (["D", "A", "A", "A", "D", "D"] + ["D"] * len(sch))[: len(sch)]
    return sch, ctype


def _bf16_pack(x):
    """f32 array -> uint16 bf16 (RNE)."""
    x = np.ascontiguousarray(x, np.float32)
    u = x.view(np.uint32)
    r = ((u >> 16) + ((u >> 15) & 1)).astype(np.uint32)
    return (r & 0xFFFF).astype(np.uint16)


def _host_negs():
    """[38, 720] f16 table: rows 0-35 -S/16, row 36 offset-cancel, row 37 a
    payload encoding (sigma0, sigma1, lehmer d2, d3, d4) of each permutation
    in the low-order score bits."""
    negs = np.zeros((38, NPERM), np.float16)
    for p in range(NPERM):
        for i in range(NUM_MODES):
            negs[i * 6 + PERMS[p, i], p] = np.float16(-1.0 / 16.0)
    negs[36, :] = np.float16(384.0)
    pr = np.arange(NPERM)
    d0 = pr // 120
    r = pr - 120 * d0
    d1 = r // 24
    r = r - 24 * d1
    d2 = r // 6
    r = r - 6 * d2
    d3 = r // 2
    d4 = r - 2 * d3
    k = PERMS[:, 0] * 256 + PERMS[:, 1] * 32 + d2 * 8 + d3 * 2 + d4
    negs[37, :] = ((512.0 + k) * 2.0 ** -16).astype(np.float16)
    return negs


def build_nc(nsh):
    """Per-core Bass program for a shard of `nsh` agents (nsh % 256 == 0)."""
    import concourse.bacc as bacc
    import concourse.mybir as mybir
    import concourse.tile as tile

    f32 = mybir.dt.float32
    f16 = mybir.dt.float16
    bf16 = mybir.dt.bfloat16
    u16 = mybir.dt.uint16
    u32 = mybir.dt.uint32
    i32 = mybir.dt.int32
    Alu = mybir.AluOpType
    Act = mybir.ActivationFunctionType
    AxX = mybir.AxisListType.X

    A = nsh // PPART
    assert A % 4 == 0
    NPAIR = A // 2
    G = 16 * A  # agents per 16-partition group
    UNITS = G * NUM_MODES  # 4-elem units per partition in gather layout
    FREE = UNITS * 4  # bf16 elems per partition

    # chunking of the back half (gather + smooth chunks; smaller at the tail)
    GCH, CTYPE0 = _chunks(A)
    SCH = GCH
    # decode segments (block ranges; must cover gather-chunk boundaries)
    DSEG = [(0, A)]
    # lnT/lpT dma chunks
    DCH = [(0, 8), (8, 16), (16, A)] if A == 20 else [(0, A)]

    nc = bacc.Bacc(None, target_bir_lowering=False, debug=False)

    # f16/bf16 payloads are shipped as f32-typed words (bitcast on SBUF side)
    # negs and tds are packed in one dram tensor: [negs | tds] per row
    td_d = nc.declare_dram_parameter("ngtd", [38, (NPERM + A * PPART) // 2], f32, False)
    cc_d = nc.declare_dram_parameter("cconst", [PPART, A], f32, False)
    lnT_d = nc.declare_dram_parameter("lnT", [PPART, FREE // 2], f32, False)
    lpT_d = nc.declare_dram_parameter("lpT", [PPART, FREE // 2], f32, False)
    NACC = 2 * len(SCH)
    out_d = nc.declare_dram_parameter("partials", [PPART, NACC], f32, True)

    with tile.TileContext(nc) as tc:
        with (
            tc.tile_pool(name="cst", bufs=1) as cst,
            tc.tile_pool(name="big", bufs=1) as big,
            tc.tile_pool(name="sml", bufs=1) as sml,
            tc.tile_pool(name="pnm", bufs=2, space="PSUM") as pnm,
        ):
            # ---- small inputs (match-path first: they gate everything);
            # negs + first half of tds land in ONE dma so matmuls start asap ----
            ngtd = cst.tile([38, NPERM + A * PPART], f16)
            negs = ngtd[:, 0:NPERM]
            H = 4
            nc.sync.dma_start(
                ngtd[:, 0 : NPERM + H * PPART].bitcast(f32),
                td_d[:, 0 : (NPERM + H * PPART) // 2],
            )
            nc.sync.dma_start(
                ngtd[:, NPERM + H * PPART :].bitcast(f32),
                td_d[:, (NPERM + H * PPART) // 2 :],
            )

            def tds(a):
                return ngtd[:, NPERM + a * PPART : NPERM + (a + 1) * PPART]

            cconst = cst.tile([PPART, A], f32)
            nc.sync.dma_start(cconst[:], cc_d[:])

            # ---- big trajectory tensors (gather layout, bf16), chunked so
            # early gathers / subs can start before the whole load lands ----
            lnT = big.tile([PPART, UNITS, 4], bf16)
            lpT = big.tile([PPART, UNITS, 4], bf16)
            UB = UNITS // A  # units per block (96)
            for lo_, hi_ in DCH:
                nc.sync.dma_start(
                    lnT[:, lo_ * UB : hi_ * UB, :]
                    .rearrange("p a b -> p (a b)")
                    .bitcast(f32),
                    lnT_d[:, lo_ * UB * 2 : hi_ * UB * 2],
                )
            for lo_, hi_ in DCH:
                nc.sync.dma_start(
                    lpT[:, lo_ * UB : hi_ * UB, :]
                    .rearrange("p a b -> p (a b)")
                    .bitcast(f32),
                    lpT_d[:, lo_ * UB * 2 : hi_ * UB * 2],
                )

            partials = sml.tile([PPART, NACC], f32)
            nc.vector.memset(partials[:], 0.0)
            # tiny dummy activation up front so the ACT table set loads
            # during the DMA phase instead of on the smooth critical path
            warm = sml.tile([PPART, 1], bf16)
            nc.vector.memset(warm[:], 0.0)
            nc.scalar.activation(warm[:], warm[:], Act.Abs, bias=0.0)
            nc.scalar.activation(warm[:], warm[:], Act.Square, bias=0.0)
            # PE p-state warmup: junk matmuls keep the PE busy while the tds
            # DMA is in flight so real matmuls run at full clock
            wmm = sml.tile([2, 64], f16)
            nc.vector.memset(wmm[:], 0.0)
            wps = pnm.tile([PPART, 2, 1024], f32, tag="nm")
            for _ in range(6):
                nc.tensor.matmul(wps[0:1, 0, 0:64], wmm[0:1, 0:1], wmm[0:1, :])
            sel = big.tile([PPART, UNITS, 4], bf16)
            dd = big.tile([PPART, FREE], bf16)
            scr = big.tile([PPART, FREE], bf16)
            mseg = sml.tile([PPART, A], f32)
            # one idx tile per decode segment so early gathers only wait on
            # their own segment's writer
            idxt = [
                sml.tile([PPART, hi_ - lo_, 6], u16, name=f"idx{i}")
                for i, (lo_, hi_) in enumerate(DSEG)
            ]

            def idx_ap(a0, a1):
                for (lo_, hi_), t in zip(DSEG, idxt):
                    if lo_ <= a0 and a1 <= hi_:
                        return t[:, a0 - lo_ : a1 - lo_, :]
                raise AssertionError((a0, a1))

            # ---- match: 4 matmuls per block-pair -> PSUM [128, 2, 1024];
            # DVE folds 720->360 out of PSUM (gpsimd cannot read PSUM), the
            # pool folds 360->90 in SBUF, DVE tensor_reduce finishes ----
            def match_pair(j):
                # gpsimd cannot run min/max ops and only one PSUM input is
                # allowed per instruction, so the whole 720-way reduction is
                # a single DVE tensor_reduce straight out of PSUM.
                nm = pnm.tile([PPART, 2, 1024], f32, tag="nm")
                for h in (0, 1):
                    a = 2 * j + h
                    nc.tensor.matmul(nm[:, h, 0:512], tds(a), negs[:, 0:512])
                    nc.tensor.matmul(nm[:, h, 512:NPERM], tds(a), negs[:, 512:NPERM])
                nc.vector.tensor_reduce(
                    mseg[:, 2 * j : 2 * j + 2], nm[:, :, 0:NPERM], AxX, Alu.max
                )

            def decode_stages(lo_, hi_, k, eng=None):
                """Payload extraction + short Lehmer adjust for [lo_, hi_).

                The max value carries (sigma0, sigma1, d2, d3, d4) packed in
                its low bits.  Returns a list of stage thunks so the issue
                order can interleave them with the match stream (DVE runs
                in-order; each stage's ops slot into reduce gaps).
                """
                if eng is None:
                    eng = nc.vector
                L = hi_ - lo_
                vm = mseg[:, lo_:hi_]
                sig = sml.tile([PPART, L, 6], f32, name=f"sig{k}")
                st = {}

                def sslice(i):
                    return sig[:, :, i : i + 1].rearrange("p a x -> p (a x)")

                def tt(op, x, y, nm_, out=None):
                    if out is None:
                        r = sml.tile([PPART, L], f32, name=f"{nm_}_{k}")
                        out = r[:]
                    eng.tensor_tensor(out, x, y, op)
                    return out

                def geadd(v, sv, nm_, out=None):
                    ge = tt(Alu.is_ge, v, sv, nm_ + "g")
                    return tt(Alu.add, v, ge, nm_ + "a", out=out)

                def stage0():
                    c1 = sml.tile([PPART, L], f32, name=f"c1_{k}")
                    nc.vector.tensor_scalar(c1[:], vm, EXT_C, None, Alu.add)
                    negio = sml.tile([PPART, L], f32, name=f"negio{k}")
                    nc.vector.scalar_tensor_tensor(
                        negio[:], c1[:], EXT_C, vm, Alu.subtract, Alu.subtract
                    )
                    nf = sml.tile([PPART, L], i32, name=f"nf{k}")
                    nc.vector.tensor_scalar(
                        nf[:], negio[:], -65536.0, -512.0, Alu.mult, Alu.add
                    )
                    # bit fields: i32-only on DVE (TSP bitVec ops cannot cast)
                    fi = sml.tile([PPART, L, 5], i32, name=f"fi{k}")
                    for j, (shift, mask) in enumerate(
                        ((8, 7), (5, 7), (3, 3), (1, 3), (0, 1))
                    ):
                        nc.vector.tensor_scalar(
                            fi[:, :, j], nf[:], shift, mask,
                            Alu.logical_shift_right, Alu.bitwise_and,
                        )
                    ff = sml.tile([PPART, L, 5], f32, name=f"ff{k}")
                    nc.vector.tensor_copy(ff[:], fi[:])
                    nc.vector.tensor_copy(sig[:, :, 0:2], ff[:, :, 0:2])
                    for n_, i_ in (("s0", 0), ("s1", 1), ("d2", 2), ("d3", 3), ("d4", 4)):
                        st[n_] = ff[:, :, i_ : i_ + 1].rearrange("p a x -> p (a x)")

                def ts(op, x, sc, nm_):
                    r = sml.tile([PPART, L], f32, name=f"{nm_}_{k}")
                    nc.vector.tensor_scalar(r[:], x, sc, None, op)
                    return r[:]

                def rankins(d, ws, nm_, out):
                    # v = d + sum_j [w_j <= d + j - 1]  (w sorted ascending):
                    # shallow parallel form of the branchless rank-insert
                    dps = [d] + [
                        ts(Alu.add, d, float(j), f"{nm_}dp{j}")
                        for j in range(1, len(ws))
                    ]
                    es = [
                        tt(Alu.is_le, w_, dp_, f"{nm_}e{j}")
                        for j, (w_, dp_) in enumerate(zip(ws, dps))
                    ]
                    v = tt(Alu.add, d, es[0], f"{nm_}a0")
                    for j, e_ in enumerate(es[1:], 1):
                        o = out if j == len(es) - 1 else None
                        v = tt(Alu.add, v, e_, f"{nm_}a{j}", out=o)
                    return v

                def stage1():
                    lo01 = tt(Alu.min, st["s0"], st["s1"], "lo")
                    hi01 = tt(Alu.max, st["s0"], st["s1"], "hi")
                    s2 = rankins(st["d2"], [lo01, hi01], "L2", sslice(2))
                    # sorted triple of {s0, s1, s2}
                    t1 = tt(Alu.min, lo01, s2, "t1")
                    t3 = tt(Alu.max, hi01, s2, "t3")
                    m1 = tt(Alu.min, hi01, s2, "m1")
                    tmid = tt(Alu.max, lo01, m1, "tmid")
                    s3 = rankins(st["d3"], [t1, tmid, t3], "L3", sslice(3))
                    st["t1"], st["tmid"], st["t3"], st["s3"] = t1, tmid, t3, s3
                    st["p01"] = tt(Alu.add, st["s0"], st["s1"], "p01")
                    st["p23"] = tt(Alu.add, s2, s3, "p23")

                def stage2():
                    # sorted quad of {t1, tmid, t3, s3}
                    w1 = tt(Alu.min, st["t1"], st["s3"], "w1")
                    q1 = tt(Alu.max, st["t1"], st["s3"], "q1")
                    w2 = tt(Alu.min, st["tmid"], q1, "w2")
                    q2 = tt(Alu.max, st["tmid"], q1, "q2")
                    w3 = tt(Alu.min, st["t3"], q2, "w3")
                    w4 = tt(Alu.max, st["t3"], q2, "w4")
                    s4 = rankins(st["d4"], [w1, w2, w3, w4], "L4", sslice(4))
                    z1 = tt(Alu.add, st["p01"], st["p23"], "z1")
                    z2 = tt(Alu.add, z1, s4, "z2")
                    nc.vector.tensor_scalar(
                        sslice(5), z2, -1.0, 15.0, Alu.mult, Alu.add
                    )
                    # gather index values (uint32 units): 2*sigma + cconst
                    nc.vector.scalar_tensor_tensor(
                        idx_ap(lo_, hi_),
                        sig[:],
                        2.0,
                        cconst[:, lo_:hi_].unsqueeze(2).broadcast_to([PPART, L, 6]),
                        Alu.mult,
                        Alu.add,
                    )

                return [stage0, stage1, stage2]

            def gather(c):
                # 4-block chunks, data/out bitcast to uint32 (half the
                # elements); indices are chunk-local 4B-unit offsets.
                a0, a1 = GCH[c]
                m0, m1 = a0 * UB, a1 * UB
                nc.gpsimd.indirect_copy(
                    sel[:, m0:m1, :]
                    .rearrange("p a b -> p (a b)")
                    .bitcast(u32)
                    .rearrange("p (a b) -> p a b", b=2),
                    lnT[:, m0:m1, :]
                    .rearrange("p a b -> p (a b)")
                    .bitcast(u32)
                    .rearrange("p (a b) -> p a b", b=2),
                    idx_ap(a0, a1).rearrange("p a x -> p (a x)"),
                    True,
                )

            # Per-chunk identity (all chunks): sum smooth(d) = sum|d| - K/2 +
            # 0.5*sum(1-min(|d|,1))^2.  Chunk type picks the engine doing the
            # two nonlinear accum passes: "A" = ACT (Abs / Square), "D" = DVE
            # (stt max(-d,d) / stt square; trn2 DVE has no abs ALU op, but
            # (d mult -1) max d with accum is a legal scalar_tensor_tensor).
            CTYPE = CTYPE0

            def smooth(s):
                a_lo, a_hi = SCH[s]
                e_ = slice(a_lo * UB * 4, a_hi * UB * 4)
                m_ = slice(a_lo * UB, a_hi * UB)
                nc.vector.tensor_sub(
                    dd[:, e_],
                    lpT[:, m_, :].rearrange("p a b -> p (a b)"),
                    sel[:, m_, :].rearrange("p a b -> p (a b)"),
                )
                if CTYPE[s] == "A":
                    nc.scalar.activation(
                        scr[:, e_], dd[:, e_], Act.Abs, bias=0.0,
                        accum_out=partials[:, 2 * s : 2 * s + 1],
                    )
                    # m = min(|d|,1); Square(1-m) back on ACT
                    nc.vector.tensor_scalar(
                        dd[:, e_], scr[:, e_], 1.0, None, Alu.min
                    )
                    nc.scalar.activation(
                        scr[:, e_], dd[:, e_], Act.Square, bias=1.0, scale=-1.0,
                        accum_out=partials[:, 2 * s + 1 : 2 * s + 2],
                    )
                else:
                    nc.vector.scalar_tensor_tensor(
                        scr[:, e_], dd[:, e_], -1.0, dd[:, e_],
                        Alu.mult, Alu.max,
                        accum_out=partials[:, 2 * s : 2 * s + 1],
                    )
                    # m - 1 = min(|d|,1) - 1
                    nc.vector.tensor_scalar(
                        dd[:, e_], scr[:, e_], 1.0, 1.0, Alu.min, Alu.subtract
                    )
                    nc.vector.scalar_tensor_tensor(
                        scr[:, e_], dd[:, e_], 1.0, dd[:, e_],
                        Alu.mult, Alu.mult,
                        accum_out=partials[:, 2 * s + 1 : 2 * s + 2],
                    )

            # ---- issue order.  Cross-engine waits are issue-point coarse
            # (a consumer waits for the producer ENGINE's whole stream up to
            # the consumer's issue position), so: early gathers are issued
            # immediately after decode-segment-0's idx write, late gathers
            # immediately after segment 1, and all smooth work after that.
            # Decode-0 stages interleave with the trailing reduces. ----
            EARLY_G = [
                c for c, (a0, a1) in enumerate(GCH)
                if DSEG[0][0] <= a0 and a1 <= DSEG[0][1]
            ]
            NP1 = DSEG[0][1] // 2  # pairs feeding decode segment 0
            for j in range(NP1):
                match_pair(j)
            d1 = decode_stages(DSEG[0][0], DSEG[0][1], 0)
            d1[0]()
            nj = NP1
            for jr in range(1, len(d1)):
                if nj < NPAIR:
                    match_pair(nj)
                    nj += 1
                d1[jr]()
            for c in EARLY_G:
                gather(c)
            for j in range(nj, NPAIR):
                match_pair(j)
            if len(DSEG) > 1:
                for s_ in decode_stages(DSEG[1][0], DSEG[1][1], 1):
                    s_()
            for c in range(len(GCH)):
                if c not in EARLY_G:
                    gather(c)
            # gate the subs on the last decode's idx so the list scheduler
            # can't starve the decode chain (whose output the last gathers
            # wait on) with ready smooth work

            for c in range(min(3, len(GCH))):
                nc.vector.tensor_scalar(
                    dd[:, GCH[c][0] * UB * 4 : GCH[c][0] * UB * 4 + 1]
                    .bitcast(u16),
                    idxt[-1][:, DSEG[-1][1] - DSEG[-1][0] - 1, 5:6],
                    0,
                    None,
                    Alu.bitwise_and,
                )
            for c in range(len(GCH)):
                smooth(c)

            nc.sync.dma_start(out_d[:], partials[:])

    nc.finalize()
    return nc


def _prep_host(pred_past, pred_now, pad_loc, pad_loc_mask, pad_loc_target, n_pad):
    """Build all per-core host tensors (list of 8 dicts)."""
    n = pred_past.shape[1]
    nsh = n_pad // N_CORES
    A = nsh // PPART

    valid = (~pad_loc_mask).astype(np.float32)

    # full agent-major trajectories, zeroed outside valid agents
    lp = np.zeros((n_pad, NUM_MODES, TC), np.float32)
    ln = np.zeros((n_pad, NUM_MODES, TC), np.float32)
    pp = pred_past[..., :2].transpose(1, 0, 2, 3) + pad_loc.transpose(1, 0, 2)[
        :, :, None, :
    ]
    pn = pred_now[..., :2].transpose(1, 0, 2, 3) + pad_loc_target[:, None, None, :]
    pp *= valid[:, None, None, None]
    pn *= valid[:, None, None, None]
    lp[:n, :, 0:60] = pp.reshape(n, NUM_MODES, 60)
    ln[:n, :, 0:60] = pn.reshape(n, NUM_MODES, 60)

    # quantized endpoint distance matrix -> tds rows (f16, exact ints)
    qd = np.zeros((n_pad, 36), np.float16)
    dx = pp[:, :, None, T - 1, 0] - pn[:, None, :, T - 1, 0]
    dy = pp[:, :, None, T - 1, 1] - pn[:, None, :, T - 1, 1]
    dist = np.minimum(np.sqrt((dx * dx + dy * dy).astype(np.float32)), 8.0)
    qd[:n] = (1024.0 + 16.0 * dist.reshape(n, 36)).astype(np.float16)

    negs = _host_negs()
    # chunk-local uint32-unit gather bases: 2 u32 per (mode, t-slot) unit,
    # 96 units -> 192 u32 per block; base = block offset within its gather
    # chunk (chunks are NOT all 4 blocks: the tail uses 2-block chunks)
    sch, _ = _chunks(A)
    local = np.zeros(A, np.float32)
    for lo, hi in sch:
        local[lo:hi] = np.arange(hi - lo, dtype=np.float32)
    cc = 192.0 * local[None, :] + 12.0 * (
        np.arange(PPART, dtype=np.float32) % 16
    )[:, None]

    in_maps = []
    for c in range(N_CORES):
        s = slice(c * nsh, (c + 1) * nsh)
        lp_c = lp[s]  # [nsh, 6, 64]
        ln_c = ln[s]
        # [a, g, q, i, t, e] -> [g, t, a, q, i, e]
        src_ln = ln_c.reshape(A, 8, 16, NUM_MODES, 16, 4)
        lnT = src_ln.transpose(1, 4, 0, 2, 3, 5).reshape(PPART, -1)
        src_lp = lp_c.reshape(A, 8, 16, NUM_MODES, 16, 4)
        # [g, t, a, i, q, e] so free offset = ((6a+i)*16 + q)*4 + e
        lpT = src_lp.transpose(1, 4, 0, 3, 2, 5).reshape(PPART, -1)

        # ngtd [38, 720 + A*128] f16: negs table, then tds (rows 0-35 =
        # quantized dists transposed, rows 36/37 = 1.0)
        ngtd = np.ones((38, NPERM + A * PPART), np.float16)
        ngtd[:, 0:NPERM] = negs
        ngtd[0:36, NPERM:] = qd[s].reshape(A * PPART, 36).T

        in_maps.append(
            {
                "ngtd": np.ascontiguousarray(ngtd).view(np.uint16).view(np.float32),
                "cconst": cc,
                "lnT": _bf16_pack(lnT).view(np.float32),
                "lpT": _bf16_pack(lpT).view(np.float32),
            }
        )
    return in_maps, float(max(valid.sum(), 1.0)), A


_CACHE = {}
LAST_RESULT = None


def kernel(pred_past, pred_now, pad_loc, pad_loc_mask, pad_loc_target):
    global LAST_RESULT
    from concourse.bass_utils import run_bass_kernel_spmd

    pred_past = np.asarray(pred_past, np.float32)
    pred_now = np.asarray(pred_now, np.float32)
    pad_loc = np.asarray(pad_loc, np.float32)
    pad_loc_mask = np.asarray(pad_loc_mask, bool)
    pad_loc_target = np.asarray(pad_loc_target, np.float32)

    n = pred_past.shape[1]
    step = N_CORES * PPART * 2
    n_pad = ((n + step - 1) // step) * step
    nsh = n_pad // N_CORES

    in_maps, n_valid, A = _prep_host(
        pred_past, pred_now, pad_loc, pad_loc_mask, pad_loc_target, n_pad
    )

    if nsh not in _CACHE:
        _CACHE[nsh] = build_nc(nsh)
    nc = _CACHE[nsh]

    res = run_bass_kernel_spmd(nc, in_maps, list(range(N_CORES)))
    LAST_RESULT = res
    parts = np.stack([r["partials"] for r in res.results])  # [8, 128, ncols]
    sums = parts.sum(axis=(0, 1), dtype=np.float64)

    # all chunks: sum smooth(d) = sum|d| - K/2 + 0.5*sum(1-min(|d|,1))^2
    # (zero rows contribute exactly 0)
    k_cons = N_CORES * PPART * (A * 16 * NUM_MODES * 4)
    cons_sum = sums[0::2].sum() - 0.5 * k_cons + 0.5 * sums[1::2].sum()
    cons_loss = np.float32(cons_sum / (NUM_MODES * T * 2 * n_valid))

    # reg loss is a cheap pure function of two small inputs -> host
    rd = (pad_loc.transpose(1, 0, 2) - pad_loc_target[:, None, :]) * (
        ~pad_loc_mask
    ).astype(np.float32)[:, None, None]
    ra = np.abs(rd)
    rr = np.maximum(1.0 - ra, 0.0)
    reg_sum = (
        ra.sum(dtype=np.float64)
        - 0.5 * rd.size
        + 0.5 * (rr.astype(np.float64) ** 2).sum()
    )
    reg_loss = np.float32(reg_sum / (NUM_MODES * 2 * n_valid))
    return (reg_loss, cons_loss)
